# revision 19
# baseline (speedup 1.0000x reference)
"""Distributed Trainium2 Bass kernel for BrosAttention.

B=2, S=1024, H=768, NH=12, DH=64:
  q,k,v = heads(hidden @ W.T + b)
  scores = q@k^T + einsum('bnid,bijd->bnij', q, bpe)   (bpe = bbox transposed)
  probs  = softmax(scores / 8)
  out    = LN(probs@v @ Wo.T + bo + hidden)

Sharding: 8 cores = 2 batches x 4 query-row blocks of 256 rows. Each core
reads only its 64MB slice of bbox_pos_emb, computes K/V for the full
sequence of its batch (duplicated 4x, cheaper than a collective here), and
writes a disjoint [256, 768] output slice. No collectives.

Layout: transposed scores (scoresT[j, i] per head) because the bias term
q.bpe needs d on partitions; bpe arrives [j, d] and is PE-transposed with
two query rows packed per [128, j] tile. The bias matmul (lhsT = q of one
row as a [64, 12] weight) runs 4 i's concurrently in the four 32-column
groups of the PE array; bias tiles are PE-transposed again into [j, (i,n)]
and added to QK^T psum tiles via a stride-12 AP. Softmax-over-partitions
uses ones-vector matmuls; probs stay unnormalized until after P@V.
"""

import os
import sys
import numpy as np

sys.path.insert(0, "/opt/trn_rl_repo")

B, S, H, NH, DH = 2, 1024, 768, 12, 64
EPS = 1e-12
P = 128
I_CORE = S * B // 8  # 256
N_CORES = 8

_COMPILED = {}


def build_kernel(s=S, i_core=I_CORE, h=H, nh=NH, dh=DH):
    from contextlib import ExitStack
    from concourse import bacc, bass, mybir, tile

    f32 = mybir.dt.float32
    Alu = mybir.AluOpType
    Act = mybir.ActivationFunctionType
    AxisX = mybir.AxisListType.X

    SC = s // P          # 8 seq chunks
    HC = h // P          # 6 hidden chunks
    IH = i_core // 2     # 128 i's per half
    NDUO_H = IH // 4     # 32 duos per half
    JH = min(512, s)     # fp32 matmul N limit / psum bank
    NJH = s // JH        # 2
    HP = nh // 2         # 6 head pairs
    VH = h // 2          # 384

    nc = bacc.Bacc(None, target_bir_lowering=False, debug=False)

    d_hidF = nc.declare_dram_parameter("hid_full", [SC, P, h], f32, isOutput=False)
    d_hidR = nc.declare_dram_parameter("hid_rows", [i_core // P, P, h], f32, isOutput=False)
    d_bpe = nc.declare_dram_parameter("bpe", [i_core, SC, P, dh], f32, isOutput=False)
    d_W = {w: nc.declare_dram_parameter(w, [HC, P, h], f32, isOutput=False)
           for w in ("Wq", "Wk", "Wv", "Wo")}
    d_b = {bn: nc.declare_dram_parameter(bn, [1, h], f32, isOutput=False)
           for bn in ("bq", "bk", "bv", "bo", "ln_gamma", "ln_beta")}
    d_ident = nc.declare_dram_parameter("ident", [P, P], f32, isOutput=False)
    d_out = nc.declare_dram_parameter("out", [i_core // P, P, h], f32, isOutput=True)

    with tile.TileContext(nc) as tc, ExitStack() as ctx:
        # ------------- long-lived pools -------------
        const_p = ctx.enter_context(tc.tile_pool(name="const", bufs=1))
        stat_p = ctx.enter_context(tc.tile_pool(name="stat", bufs=1))
        ps128 = ctx.enter_context(
            tc.tile_pool(name="ps128", bufs=2, space=bass.MemorySpace.PSUM))
        ps512 = ctx.enter_context(
            tc.tile_pool(name="ps512", bufs=2, space=bass.MemorySpace.PSUM))
        psB = ctx.enter_context(
            tc.tile_pool(name="psB", bufs=2, space=bass.MemorySpace.PSUM))
        psS = ctx.enter_context(
            tc.tile_pool(name="psS", bufs=1, space=bass.MemorySpace.PSUM))
        psC = ctx.enter_context(
            tc.tile_pool(name="psC", bufs=1, space=bass.MemorySpace.PSUM))

        # ------------- constants -------------
        ident = const_p.tile([P, P], f32)
        nc.sync.dma_start(ident[:], d_ident[:])
        ones_col = const_p.tile([P, 1], f32)
        nc.vector.memset(ones_col[:], 1.0)
        ones_row = const_p.tile([1, s], f32)
        nc.vector.memset(ones_row[:], 1.0)
        eps_t = const_p.tile([P, 1], f32)
        nc.vector.memset(eps_t[:], EPS)
        zrow = const_p.tile([1, P], f32)
        nc.vector.memset(zrow[:], 0.0)
        b_sb = {}
        for bn in ("bq", "bk", "bv", "bo", "ln_gamma", "ln_beta"):
            b_sb[bn] = const_p.tile([1, h], f32, name=f"bias_{bn}")
            nc.sync.dma_start(b_sb[bn][:], d_b[bn][:])

        bcast = {}
        for bn in ("ln_gamma", "ln_beta"):
            t = stat_p.tile([P, h], f32, name=f"bcast_{bn}")
            for c in range(HC):
                pbx = ps128.tile([P, P], f32, name="pt")
                nc.tensor.matmul(pbx[:], ones_row[:, 0:P],
                                 b_sb[bn][:, c * P:(c + 1) * P])
                nc.scalar.copy(t[:, c * P:(c + 1) * P], pbx[:])
            bcast[bn] = t

        # long-lived activations
        hidR = stat_p.tile([P, i_core // P, h], f32)
        nc.sync.dma_start(hidR[:], d_hidR[:].transpose([1, 0, 2]))
        WoT = stat_p.tile([P, HC, h], f32)
        qT128 = stat_p.tile([P, nh, i_core], f32)   # q[n,i,:] at both 64-halves
        kT128 = stat_p.tile([P, HP, s], f32)
        v_sb = stat_p.tile([P, SC, h], f32)

        def pe_T(dst_ap, src_ap, copy_eng):
            pt = ps128.tile([P, P], f32)
            n = src_ap.shape[-1]
            nc.tensor.transpose(pt[0:n, :], src_ap, ident[:])
            if copy_eng is nc.scalar:
                copy_eng.copy(dst_ap, pt[0:n, :])
            else:
                copy_eng.tensor_copy(dst_ap, pt[0:n, :])

        # ------------- phase 0 -------------
        with tc.tile_pool(name="early", bufs=1) as early_p, \
             tc.tile_pool(name="wnat", bufs=1) as wnat_p, \
             tc.tile_pool(name="wtw", bufs=1) as wtw_p:
            hidF = early_p.tile([P, SC, h], f32)
            nc.sync.dma_start(hidF[:], d_hidF[:].transpose([1, 0, 2]))
            hidT = early_p.tile([P, HC, s], f32)
            for rc in range(HC):
                for jc in range(SC):
                    pe_T(hidT[:, rc, jc * P:(jc + 1) * P],
                         hidF[:, jc, rc * P:(rc + 1) * P], nc.vector)
            hidRT = early_p.tile([P, HC, i_core], f32)
            for rc in range(HC):
                for ic in range(i_core // P):
                    pe_T(hidRT[:, rc, ic * P:(ic + 1) * P],
                         hidR[:, ic, rc * P:(rc + 1) * P], nc.vector)

            def load_WT(w, dst):
                wn = wnat_p.tile([P, HC, h], f32, name="wnat")
                nc.sync.dma_start(wn[:], d_W[w][:].transpose([1, 0, 2]))
                for rc in range(HC):
                    for c in range(HC):
                        pe_T(dst[:, rc, c * P:(c + 1) * P],
                             wn[:, c, rc * P:(rc + 1) * P], nc.scalar)
                return dst

            # Q projection (transposed): qT = Wq @ hidR^T + bq
            WqT = load_WT("Wq", wtw_p.tile([P, HC, h], f32, name="WT"))
            for r in range(HC):
                pq = ps512.tile([P, JH], f32, name="big")
                for kc in range(HC):
                    nc.tensor.matmul(pq[:, 0:i_core],
                                     WqT[:, kc, r * P:(r + 1) * P],
                                     hidRT[:, kc, :], start=(kc == 0), stop=False)
                nc.tensor.matmul(pq[:, 0:i_core], b_sb["bq"][:, r * P:(r + 1) * P],
                                 ones_row[:, 0:i_core], start=False, stop=True)
                for sub in range(2):
                    src = pq[sub * dh:(sub + 1) * dh, 0:i_core]
                    nc.vector.tensor_copy(qT128[0:dh, 2 * r + sub, :], src)
                    nc.vector.tensor_copy(qT128[dh:P, 2 * r + sub, :], src)

            # K projection (transposed): kT = Wk @ hid^T + bk
            WkT = load_WT("Wk", wtw_p.tile([P, HC, h], f32, name="WT"))
            for r in range(HC):
                for jh in range(NJH):
                    pk = ps512.tile([P, JH], f32, name="big")
                    for kc in range(HC):
                        nc.tensor.matmul(pk[:], WkT[:, kc, r * P:(r + 1) * P],
                                         hidT[:, kc, jh * JH:(jh + 1) * JH],
                                         start=(kc == 0), stop=False)
                    nc.tensor.matmul(pk[:], b_sb["bk"][:, r * P:(r + 1) * P],
                                     ones_row[:, 0:JH], start=False, stop=True)
                    nc.vector.tensor_copy(
                        kT128[:, r, jh * JH:(jh + 1) * JH], pk[:])

            # V projection (natural): v = hid @ Wv^T + bv
            WvT = load_WT("Wv", wtw_p.tile([P, HC, h], f32, name="WT"))
            for jc in range(SC):
                for vh in range(2):
                    pv = ps512.tile([P, JH], f32, name="big")
                    for kc in range(HC):
                        nc.tensor.matmul(pv[:, 0:VH],
                                         hidT[:, kc, jc * P:(jc + 1) * P],
                                         WvT[:, kc, vh * VH:(vh + 1) * VH],
                                         start=(kc == 0), stop=False)
                    nc.tensor.matmul(pv[:, 0:VH], ones_row[:, 0:P],
                                     b_sb["bv"][:, vh * VH:(vh + 1) * VH],
                                     start=False, stop=True)
                    nc.vector.tensor_copy(v_sb[:, jc, vh * VH:(vh + 1) * VH],
                                          pv[:, 0:VH])

            load_WT("Wo", WoT)

        # ------------- phases A+B -------------
        with tc.tile_pool(name="bpeN", bufs=2) as bpeN_p, \
             tc.tile_pool(name="bpeT", bufs=2) as bpeT_p, \
             tc.tile_pool(name="bias4", bufs=1) as bias4_p, \
             tc.tile_pool(name="biasT", bufs=1) as biasT_p, \
             tc.tile_pool(name="sm", bufs=2) as sm_p, \
             tc.tile_pool(name="ctxp", bufs=1) as ctx_p, \
             tc.tile_pool(name="yp", bufs=1) as y_p:
            for half in range(2):
                i0h = half * IH
                # biasT[j, jc, duo*48 + 12*i4 + n]
                biasT = biasT_p.tile([P, SC, NDUO_H * 48], f32)

                for duo in range(NDUO_H):
                    pb_h = [psB.tile([P, JH], f32, name="pbh") for j in range(NJH)]
                    for jh in range(NJH):
                        nc.tensor.matmul(pb_h[jh][:], zrow[:],
                                         ones_row[:, 0:JH],
                                         start=True, stop=False,
                                         skip_group_check=True)
                    for pp in range(2):
                        iA = i0h + 4 * duo + 2 * pp
                        bpeT = bpeT_p.tile([P, SC, P], f32)
                        bn2 = bpeN_p.tile([P, SC, 2, dh], f32)
                        for ii in range(2):
                            nc.sync.dma_start(bn2[:, :, ii, :],
                                              d_bpe[iA + ii].transpose([1, 0, 2]))
                        # one [128,128] transpose per jc packs (i0-d | i1-d)
                        NJC = JH // P
                        for jg in range(NJH):
                            pt4 = ps512.tile([P, JH], f32, name="big")
                            for jl in range(NJC):
                                jc = jg * NJC + jl
                                nc.tensor.transpose(
                                    pt4[:, jl * P:(jl + 1) * P],
                                    bn2[:, jc, :, :], ident[:])
                            nc.scalar.copy(
                                bpeT[:, jg * NJC:(jg + 1) * NJC, :],
                                pt4[:].rearrange("p (a b) -> p a b", a=NJC))
                        # bias matmuls: 4 i's of the duo -> 4 col groups
                        for ii in range(2):
                            cg = 32 * (2 * pp + ii)
                            i_loc = iA + ii
                            lhs = qT128[ii * dh:(ii + 1) * dh, :,
                                        i_loc:i_loc + 1]
                            for jh in range(NJH):
                                jc0 = (jh * JH) // P
                                rhs = bpeT[ii * dh:(ii + 1) * dh,
                                           jc0:jc0 + JH // P, :]
                                nc.tensor.matmul(
                                    pb_h[jh][cg:cg + nh, :], lhs, rhs,
                                    start=False, stop=(pp == 1 and ii == 1),
                                    skip_group_check=True,
                                    tile_position=(ii * dh, cg))
                    b4 = bias4_p.tile([P, s], f32)
                    for jh in range(NJH):
                        nc.vector.tensor_copy(b4[:, jh * JH:(jh + 1) * JH],
                                              pb_h[jh][:])
                    for jc in range(SC):
                        ptb = ps128.tile([P, P], f32, name="pt")
                        nc.tensor.transpose(ptb[:], b4[:, jc * P:(jc + 1) * P],
                                            ident[:])
                        nc.vector.tensor_copy(
                            biasT[:, jc, duo * 48:(duo + 1) * 48].rearrange(
                                "p (a b) -> p a b", a=4),
                            ptb[:].rearrange("p (a b) -> p a b", a=4)[:, :, 0:12])

                # ---- attention ----
                ctxT = ctx_p.tile([P, HP, IH], f32)
                for hp in range(HP):
                    pctx = psC.tile([P, IH], f32, name="pctx")
                    for sub in range(2):
                        n = 2 * hp + sub
                        probsT = sm_p.tile([P, SC, IH], f32)
                        psum_s = psS.tile([1, IH], f32)
                        for jc in range(SC):
                            pqk = ps128.tile([P, IH], f32, name="pt")
                            sb = sub * dh
                            nc.tensor.matmul(pqk[:],
                                             kT128[sb:sb + dh, hp, jc * P:(jc + 1) * P],
                                             qT128[sb:sb + dh, n, i0h:i0h + IH])
                            sE = sm_p.tile([P, IH], f32)
                            nc.vector.tensor_tensor(
                                sE[:], pqk[:],
                                biasT[:, jc, n:n + 12 * (IH - 1) + 1:12], Alu.add)
                            nc.scalar.activation(probsT[:, jc, :], sE[:],
                                                 Act.Exp, scale=0.125)
                            nc.tensor.matmul(psum_s[:], ones_col[:],
                                             probsT[:, jc, :],
                                             start=(jc == 0), stop=(jc == SC - 1),
                                             skip_group_check=True)
                        rec = sm_p.tile([1, IH], f32)
                        nc.vector.reciprocal(rec[:], psum_s[:])
                        prec = ps128.tile([P, IH], f32, name="pt")
                        nc.tensor.matmul(prec[0:dh, :], ones_row[:, 0:dh], rec[:])
                        recB = sm_p.tile([dh, IH], f32)
                        nc.scalar.copy(recB[:], prec[0:dh, :])
                        for jc in range(SC):
                            nc.tensor.matmul(
                                pctx[sub * dh:(sub + 1) * dh, :],
                                v_sb[:, jc, n * dh:(n + 1) * dh],
                                probsT[:, jc, :],
                                start=(jc == 0), stop=(jc == SC - 1),
                                tile_position=(0, sub * dh),
                                skip_group_check=True)
                        nc.vector.tensor_tensor(
                            pctx[sub * dh:(sub + 1) * dh, :],
                            pctx[sub * dh:(sub + 1) * dh, :],
                            recB[:], Alu.mult)
                    nc.scalar.copy(ctxT[:, hp, :], pctx[:])

                # ---- O-proj + residual + LN ----
                pys = [ps512.tile([P, VH], f32, name="big") for j in range(2)]
                for vh in range(2):
                    for kc in range(HC):
                        nc.tensor.matmul(pys[vh][:], ctxT[:, kc, :],
                                         WoT[:, kc, vh * VH:(vh + 1) * VH],
                                         start=(kc == 0), stop=False)
                    nc.tensor.matmul(pys[vh][:], ones_row[:, 0:P],
                                     b_sb["bo"][:, vh * VH:(vh + 1) * VH],
                                     start=False, stop=True)
                y = y_p.tile([P, h], f32)
                for vh in range(2):
                    nc.vector.tensor_tensor(y[:, vh * VH:(vh + 1) * VH],
                                            pys[vh][:],
                                            hidR[:, half, vh * VH:(vh + 1) * VH],
                                            Alu.add)
                mu = y_p.tile([P, 1], f32)
                nc.vector.tensor_reduce(mu[:], y[:], AxisX, Alu.add)
                nc.vector.tensor_scalar(mu[:], mu[:], 1.0 / h, None, Alu.mult)
                yc = y_p.tile([P, h], f32)
                nc.vector.tensor_scalar(yc[:], y[:], mu[:], None, Alu.subtract)
                ssq = y_p.tile([P, 1], f32)
                nc.scalar.activation(y[:], yc[:], Act.Square, accum_out=ssq[:])
                std = y_p.tile([P, 1], f32)
                nc.scalar.activation(std[:], ssq[:], Act.Sqrt,
                                     scale=1.0 / h, bias=eps_t[:])
                rstd = y_p.tile([P, 1], f32)
                nc.vector.reciprocal(rstd[:], std[:])
                o1 = y_p.tile([P, h], f32)
                nc.vector.tensor_scalar(o1[:], yc[:], rstd[:], None, Alu.mult)
                nc.vector.tensor_tensor(o1[:], o1[:], bcast["ln_gamma"][:], Alu.mult)
                nc.vector.tensor_tensor(o1[:], o1[:], bcast["ln_beta"][:], Alu.add)
                nc.sync.dma_start(d_out[half], o1[:])

    nc.compile()
    return nc


def _shard_inputs(inputs):
    hs = np.ascontiguousarray(np.asarray(inputs["hidden_states"]), dtype=np.float32)
    bpe = np.asarray(inputs["bbox_pos_emb"])
    ident = np.eye(P, dtype=np.float32)
    in_maps = []
    for c in range(N_CORES):
        b = c // 4
        q0 = (c % 4) * I_CORE
        m = {
            "hid_full": np.ascontiguousarray(hs[b].reshape(S // P, P, H)),
            "hid_rows": np.ascontiguousarray(
                hs[b, q0:q0 + I_CORE].reshape(I_CORE // P, P, H)),
            "bpe": np.ascontiguousarray(
                bpe[q0:q0 + I_CORE, :, b, :], dtype=np.float32).reshape(
                    I_CORE, S // P, P, DH),
            "ident": ident,
        }
        for w in ("Wq", "Wk", "Wv", "Wo"):
            m[w] = np.ascontiguousarray(
                np.asarray(inputs[w], dtype=np.float32).reshape(H // P, P, H))
        for bn in ("bq", "bk", "bv", "bo", "ln_gamma", "ln_beta"):
            m[bn] = np.ascontiguousarray(
                np.asarray(inputs[bn], dtype=np.float32).reshape(1, H))
        in_maps.append(m)
    return in_maps


def _install_ntff_shim():
    """The agent image's antenv lacks axon_hooks; recreate the NTFF profile
    hook via ctypes against libaxon_pjrt.so so trace=True yields
    exec_time_ns + a perfetto trace."""
    import sys as _sys
    if "antenv.axon_hooks" in _sys.modules:
        return
    import types, ctypes, contextlib
    so_path = "/opt/axon/libaxon_pjrt.so"
    mod = types.ModuleType("antenv.axon_hooks")
    _state = {}

    def get_axon_ntff_profile_hook():
        if "hook" in _state:
            return _state["hook"]
        try:
            lib = ctypes.CDLL(so_path)
            if not hasattr(lib, "axon_start_nrt_profile"):
                _state["hook"] = None
                return None
            lib.axon_start_nrt_profile.argtypes = [
                ctypes.POINTER(ctypes.c_int64), ctypes.c_size_t]
            lib.axon_start_nrt_profile.restype = ctypes.c_int64
            lib.axon_stop_nrt_profile.argtypes = [ctypes.c_char_p]
            lib.axon_stop_nrt_profile.restype = ctypes.c_int64
        except OSError:
            _state["hook"] = None
            return None

        @contextlib.contextmanager
        def _hook(output_dir, device_ids):
            import jax
            jax.devices()
            if device_ids:
                ids = (ctypes.c_int64 * len(device_ids))(*device_ids)
                rc = lib.axon_start_nrt_profile(ids, len(device_ids))
            else:
                rc = lib.axon_start_nrt_profile(None, 0)
            if rc != 0:
                raise RuntimeError(f"axon_start_nrt_profile rc={rc}")
            try:
                yield
            finally:
                n = lib.axon_stop_nrt_profile(str(output_dir).encode())
                print(f"ntff profile: {n} file(s) written to {output_dir}")

        _state["hook"] = _hook
        return _hook

    mod.get_axon_ntff_profile_hook = get_axon_ntff_profile_hook
    _sys.modules["antenv.axon_hooks"] = mod


def kernel(**inputs):
    from concourse.bass_utils import run_bass_kernel_spmd

    if os.environ.get("BASS_KERNEL_TRACE"):
        _install_ntff_shim()
        import concourse.bass_utils as _bu
        _bu.upload_artifacts = lambda tmpdir: f"file://{tmpdir}"

    if "nc" not in _COMPILED:
        _COMPILED["nc"] = build_kernel()
    nc = _COMPILED["nc"]
    in_maps = _shard_inputs(inputs)
    res = run_bass_kernel_spmd(nc, in_maps, core_ids=list(range(N_CORES)),
                               trace=bool(os.environ.get("BASS_KERNEL_TRACE")))
    _COMPILED["last_result"] = res
    out = np.zeros((B, S, H), dtype=np.float32)
    for c in range(N_CORES):
        b = c // 4
        q0 = (c % 4) * I_CORE
        out[b, q0:q0 + I_CORE] = np.asarray(
            res.results[c]["out"]).reshape(I_CORE, H)
    return out


# revision 20
# speedup vs baseline: 1.2627x; 1.2627x over previous
"""Distributed Trainium2 Bass kernel for BrosAttention.

B=2, S=1024, H=768, NH=12, DH=64:
  q,k,v = heads(hidden @ W.T + b)
  scores = q@k^T + einsum('bnid,bijd->bnij', q, bpe)   (bpe = bbox transposed)
  probs  = softmax(scores / 8)
  out    = LN(probs@v @ Wo.T + bo + hidden)

Sharding: 8 cores = 2 batches x 4 query-row blocks of 256 rows. Each core
reads only its 64MB slice of bbox_pos_emb, computes K/V for the full
sequence of its batch (duplicated 4x, cheaper than a collective here), and
writes a disjoint [256, 768] output slice. No collectives.

Layout: transposed scores (scoresT[j, i] per head) because the bias term
q.bpe needs d on partitions; bpe arrives [j, d] and is PE-transposed with
two query rows packed per [128, j] tile. The bias matmul (lhsT = q of one
row as a [64, 12] weight) runs 4 i's concurrently in the four 32-column
groups of the PE array; bias tiles are PE-transposed again into [j, (i,n)]
and added to QK^T psum tiles via a stride-12 AP. Softmax-over-partitions
uses ones-vector matmuls; probs stay unnormalized until after P@V.
"""

import os
import sys
import numpy as np

sys.path.insert(0, "/opt/trn_rl_repo")

B, S, H, NH, DH = 2, 1024, 768, 12, 64
EPS = 1e-12
P = 128
I_CORE = S * B // 8  # 256
N_CORES = 8

_COMPILED = {}


def build_kernel(s=S, i_core=I_CORE, h=H, nh=NH, dh=DH):
    from contextlib import ExitStack
    from concourse import bacc, bass, mybir, tile

    f32 = mybir.dt.float32
    bf16 = mybir.dt.bfloat16
    Alu = mybir.AluOpType
    Act = mybir.ActivationFunctionType
    AxisX = mybir.AxisListType.X

    SC = s // P          # 8 seq chunks
    HC = h // P          # 6 hidden chunks
    IH = i_core // 2     # 128 i's per half
    NDUO_H = IH // 4     # 32 duos per half
    JH = min(512, s)     # fp32 matmul N limit / psum bank
    NJH = s // JH        # 2
    HP = nh // 2         # 6 head pairs
    VH = h // 2          # 384

    nc = bacc.Bacc(None, target_bir_lowering=False, debug=False)

    d_hidF = nc.declare_dram_parameter("hid_full", [SC, P, h], f32, isOutput=False)
    d_hidR = nc.declare_dram_parameter("hid_rows", [i_core // P, P, h], f32, isOutput=False)
    d_bpe = nc.declare_dram_parameter("bpe", [i_core, SC, P, dh], f32, isOutput=False)
    d_W = {w: nc.declare_dram_parameter(w, [HC, P, h], f32, isOutput=False)
           for w in ("Wq", "Wk", "Wv", "Wo")}
    d_b = {bn: nc.declare_dram_parameter(bn, [1, h], f32, isOutput=False)
           for bn in ("bq", "bk", "bv", "bo", "ln_gamma", "ln_beta")}
    d_ident = nc.declare_dram_parameter("ident", [P, P], f32, isOutput=False)
    d_out = nc.declare_dram_parameter("out", [i_core // P, P, h], f32, isOutput=True)

    with tile.TileContext(nc) as tc, ExitStack() as ctx:
        # ------------- long-lived pools -------------
        const_p = ctx.enter_context(tc.tile_pool(name="const", bufs=1))
        stat_p = ctx.enter_context(tc.tile_pool(name="stat", bufs=1))
        ps128 = ctx.enter_context(
            tc.tile_pool(name="ps128", bufs=2, space=bass.MemorySpace.PSUM))
        ps512 = ctx.enter_context(
            tc.tile_pool(name="ps512", bufs=2, space=bass.MemorySpace.PSUM))
        psB = ctx.enter_context(
            tc.tile_pool(name="psB", bufs=2, space=bass.MemorySpace.PSUM))
        psS = ctx.enter_context(
            tc.tile_pool(name="psS", bufs=1, space=bass.MemorySpace.PSUM))
        psC = ctx.enter_context(
            tc.tile_pool(name="psC", bufs=1, space=bass.MemorySpace.PSUM))

        # ------------- constants -------------
        ident = const_p.tile([P, P], f32)
        nc.sync.dma_start(ident[:], d_ident[:])
        ones_col = const_p.tile([P, 1], f32)
        nc.vector.memset(ones_col[:], 1.0)
        ones_row = const_p.tile([1, s], f32)
        nc.vector.memset(ones_row[:], 1.0)
        eps_t = const_p.tile([P, 1], f32)
        nc.vector.memset(eps_t[:], EPS)
        zrow = const_p.tile([1, P], bf16)
        nc.vector.memset(zrow[:], 0.0)
        ident_bf = const_p.tile([P, P], bf16)
        nc.vector.tensor_copy(ident_bf[:], ident[:])
        ones_col_bf = const_p.tile([P, 1], bf16)
        nc.vector.memset(ones_col_bf[:], 1.0)
        ones_row_bf = const_p.tile([1, s], bf16)
        nc.vector.memset(ones_row_bf[:], 1.0)
        b_sb = {}
        b_bf = {}
        for bn in ("bq", "bk", "bv", "bo", "ln_gamma", "ln_beta"):
            b_sb[bn] = const_p.tile([1, h], f32, name=f"bias_{bn}")
            nc.sync.dma_start(b_sb[bn][:], d_b[bn][:])
            b_bf[bn] = const_p.tile([1, h], bf16, name=f"biasbf_{bn}")
            nc.vector.tensor_copy(b_bf[bn][:], b_sb[bn][:])

        bcast = {}
        for bn in ("ln_gamma", "ln_beta"):
            t = stat_p.tile([P, h], f32, name=f"bcast_{bn}")
            for c in range(HC):
                pbx = ps128.tile([P, P], f32, name="pt")
                nc.tensor.matmul(pbx[:], ones_row[:, 0:P],
                                 b_sb[bn][:, c * P:(c + 1) * P])
                nc.scalar.copy(t[:, c * P:(c + 1) * P], pbx[:])
            bcast[bn] = t

        # long-lived activations
        hidR = stat_p.tile([P, i_core // P, h], f32)
        nc.sync.dma_start(hidR[:], d_hidR[:].transpose([1, 0, 2]))
        WoT = stat_p.tile([P, HC, h], bf16)
        qT128 = stat_p.tile([P, nh, i_core], bf16)  # q[n,i,:] at both 64-halves
        kT128 = stat_p.tile([P, HP, s], bf16)
        v_sb = stat_p.tile([P, SC, h], bf16)

        def pe_T(dst_ap, src_ap, copy_eng):
            bf = src_ap.dtype == bf16
            pt = ps128.tile([P, P], bf16 if bf else f32, name="pt")
            n = src_ap.shape[-1]
            nc.tensor.transpose(pt[0:n, :], src_ap,
                                ident_bf[:] if bf else ident[:])
            if copy_eng is nc.scalar:
                copy_eng.copy(dst_ap, pt[0:n, :])
            else:
                copy_eng.tensor_copy(dst_ap, pt[0:n, :])

        # ------------- phase 0 -------------
        with tc.tile_pool(name="early", bufs=1) as early_p, \
             tc.tile_pool(name="wnat", bufs=1) as wnat_p, \
             tc.tile_pool(name="wtw", bufs=1) as wtw_p:
            hidF = early_p.tile([P, SC, h], bf16)
            nc.gpsimd.dma_start(hidF[:], d_hidF[:].transpose([1, 0, 2]))
            hidT = early_p.tile([P, HC, s], bf16)
            for rc in range(HC):
                for jc in range(SC):
                    pe_T(hidT[:, rc, jc * P:(jc + 1) * P],
                         hidF[:, jc, rc * P:(rc + 1) * P], nc.vector)
            hidRT = early_p.tile([P, HC, i_core], bf16)
            for rc in range(HC):
                for ic in range(i_core // P):
                    pe_T(hidRT[:, rc, ic * P:(ic + 1) * P],
                         hidR[:, ic, rc * P:(rc + 1) * P], nc.vector)

            def load_WT(w, dst):
                wn = wnat_p.tile([P, HC, h], bf16, name="wnat")
                nc.gpsimd.dma_start(wn[:], d_W[w][:].transpose([1, 0, 2]))
                for rc in range(HC):
                    for c in range(HC):
                        pe_T(dst[:, rc, c * P:(c + 1) * P],
                             wn[:, c, rc * P:(rc + 1) * P], nc.scalar)
                return dst

            # Q projection (transposed): qT = Wq @ hidR^T + bq
            WqT = load_WT("Wq", wtw_p.tile([P, HC, h], bf16, name="WT"))
            for r in range(HC):
                pq = ps512.tile([P, JH], f32, name="big")
                for kc in range(HC):
                    nc.tensor.matmul(pq[:, 0:i_core],
                                     WqT[:, kc, r * P:(r + 1) * P],
                                     hidRT[:, kc, :], start=(kc == 0), stop=False)
                nc.tensor.matmul(pq[:, 0:i_core], b_bf["bq"][:, r * P:(r + 1) * P],
                                 ones_row_bf[:, 0:i_core], start=False, stop=True)
                for sub in range(2):
                    src = pq[sub * dh:(sub + 1) * dh, 0:i_core]
                    nc.vector.tensor_copy(qT128[0:dh, 2 * r + sub, :], src)
                    nc.vector.tensor_copy(qT128[dh:P, 2 * r + sub, :], src)

            # K projection (transposed): kT = Wk @ hid^T + bk
            WkT = load_WT("Wk", wtw_p.tile([P, HC, h], bf16, name="WT"))
            for r in range(HC):
                for jh in range(NJH):
                    pk = ps512.tile([P, JH], f32, name="big")
                    for kc in range(HC):
                        nc.tensor.matmul(pk[:], WkT[:, kc, r * P:(r + 1) * P],
                                         hidT[:, kc, jh * JH:(jh + 1) * JH],
                                         start=(kc == 0), stop=False)
                    nc.tensor.matmul(pk[:], b_bf["bk"][:, r * P:(r + 1) * P],
                                     ones_row_bf[:, 0:JH], start=False, stop=True)
                    nc.vector.tensor_copy(
                        kT128[:, r, jh * JH:(jh + 1) * JH], pk[:])

            # V projection (natural): v = hid @ Wv^T + bv
            WvT = load_WT("Wv", wtw_p.tile([P, HC, h], bf16, name="WT"))
            for jc in range(SC):
                for vh in range(2):
                    pv = ps512.tile([P, JH], f32, name="big")
                    for kc in range(HC):
                        nc.tensor.matmul(pv[:, 0:VH],
                                         hidT[:, kc, jc * P:(jc + 1) * P],
                                         WvT[:, kc, vh * VH:(vh + 1) * VH],
                                         start=(kc == 0), stop=False)
                    nc.tensor.matmul(pv[:, 0:VH], ones_row_bf[:, 0:P],
                                     b_bf["bv"][:, vh * VH:(vh + 1) * VH],
                                     start=False, stop=True)
                    nc.vector.tensor_copy(v_sb[:, jc, vh * VH:(vh + 1) * VH],
                                          pv[:, 0:VH])

            load_WT("Wo", WoT)

        # ------------- phases A+B -------------
        with tc.tile_pool(name="bpeN", bufs=2) as bpeN_p, \
             tc.tile_pool(name="bpeT", bufs=2) as bpeT_p, \
             tc.tile_pool(name="bias4", bufs=1) as bias4_p, \
             tc.tile_pool(name="biasT", bufs=1) as biasT_p, \
             tc.tile_pool(name="sm", bufs=2) as sm_p, \
             tc.tile_pool(name="ctxp", bufs=1) as ctx_p, \
             tc.tile_pool(name="yp", bufs=1) as y_p:
            for half in range(2):
                i0h = half * IH
                # biasT[j, jc, duo*48 + 12*i4 + n]
                biasT = biasT_p.tile([P, SC, NDUO_H * 48], bf16)

                for duo in range(NDUO_H):
                    pb_h = [psB.tile([P, JH], f32, name="pbh") for j in range(NJH)]
                    for jh in range(NJH):
                        nc.tensor.matmul(pb_h[jh][:], zrow[:],
                                         ones_row_bf[:, 0:JH],
                                         start=True, stop=False,
                                         skip_group_check=True)
                    for pp in range(2):
                        iA = i0h + 4 * duo + 2 * pp
                        bpeT = bpeT_p.tile([P, SC, P], bf16)
                        bn2 = bpeN_p.tile([P, SC, 2, dh], bf16)
                        for ii in range(2):
                            nc.gpsimd.dma_start(bn2[:, :, ii, :],
                                              d_bpe[iA + ii].transpose([1, 0, 2]))
                        # one [128,128] transpose per jc packs (i0-d | i1-d)
                        NJC = JH // P
                        for jg in range(NJH):
                            pt4 = ps512.tile([P, JH], bf16, name="big")
                            for jl in range(NJC):
                                jc = jg * NJC + jl
                                nc.tensor.transpose(
                                    pt4[:, jl * P:(jl + 1) * P],
                                    bn2[:, jc, :, :], ident_bf[:])
                            nc.scalar.copy(
                                bpeT[:, jg * NJC:(jg + 1) * NJC, :],
                                pt4[:].rearrange("p (a b) -> p a b", a=NJC))
                        # bias matmuls: 4 i's of the duo -> 4 col groups
                        for ii in range(2):
                            cg = 32 * (2 * pp + ii)
                            i_loc = iA + ii
                            lhs = qT128[ii * dh:(ii + 1) * dh, :,
                                        i_loc:i_loc + 1]
                            for jh in range(NJH):
                                jc0 = (jh * JH) // P
                                rhs = bpeT[ii * dh:(ii + 1) * dh,
                                           jc0:jc0 + JH // P, :]
                                nc.tensor.matmul(
                                    pb_h[jh][cg:cg + nh, :], lhs, rhs,
                                    start=False, stop=(pp == 1 and ii == 1),
                                    skip_group_check=True,
                                    tile_position=(ii * dh, cg))
                    b4 = bias4_p.tile([P, s], bf16)
                    for jh in range(NJH):
                        nc.vector.tensor_copy(b4[:, jh * JH:(jh + 1) * JH],
                                              pb_h[jh][:])
                    for jc in range(SC):
                        ptb = ps128.tile([P, P], bf16, name="pt")
                        nc.tensor.transpose(ptb[:], b4[:, jc * P:(jc + 1) * P],
                                            ident_bf[:])
                        nc.vector.tensor_copy(
                            biasT[:, jc, duo * 48:(duo + 1) * 48].rearrange(
                                "p (a b) -> p a b", a=4),
                            ptb[:].rearrange("p (a b) -> p a b", a=4)[:, :, 0:12])

                # ---- attention ----
                ctxT = ctx_p.tile([P, HP, IH], bf16)
                for hp in range(HP):
                    pctx = psC.tile([P, IH], f32, name="pctx")
                    for sub in range(2):
                        n = 2 * hp + sub
                        probsT = sm_p.tile([P, SC, IH], bf16)
                        psum_s = psS.tile([1, IH], f32)
                        for jc in range(SC):
                            pqk = ps128.tile([P, IH], f32, name="pt")
                            sb = sub * dh
                            nc.tensor.matmul(pqk[:],
                                             kT128[sb:sb + dh, hp, jc * P:(jc + 1) * P],
                                             qT128[sb:sb + dh, n, i0h:i0h + IH])
                            sE = sm_p.tile([P, IH], f32)
                            nc.vector.tensor_tensor(
                                sE[:], pqk[:],
                                biasT[:, jc, n:n + 12 * (IH - 1) + 1:12], Alu.add)
                            nc.scalar.activation(probsT[:, jc, :], sE[:],
                                                 Act.Exp, scale=0.125)
                            nc.tensor.matmul(psum_s[:], ones_col_bf[:],
                                             probsT[:, jc, :],
                                             start=(jc == 0), stop=(jc == SC - 1),
                                             skip_group_check=True)
                        rec = sm_p.tile([1, IH], f32)
                        nc.vector.reciprocal(rec[:], psum_s[:])
                        prec = ps128.tile([P, IH], f32, name="pt")
                        nc.tensor.matmul(prec[0:dh, :], ones_row[:, 0:dh], rec[:])
                        recB = sm_p.tile([dh, IH], f32)
                        nc.scalar.copy(recB[:], prec[0:dh, :])
                        for jc in range(SC):
                            nc.tensor.matmul(
                                pctx[sub * dh:(sub + 1) * dh, :],
                                v_sb[:, jc, n * dh:(n + 1) * dh],
                                probsT[:, jc, :],
                                start=(jc == 0), stop=(jc == SC - 1),
                                tile_position=(0, sub * dh),
                                skip_group_check=True)
                        nc.vector.tensor_tensor(
                            pctx[sub * dh:(sub + 1) * dh, :],
                            pctx[sub * dh:(sub + 1) * dh, :],
                            recB[:], Alu.mult)
                    nc.scalar.copy(ctxT[:, hp, :], pctx[:])

                # ---- O-proj + residual + LN ----
                pys = [ps512.tile([P, VH], f32, name="big") for j in range(2)]
                for vh in range(2):
                    for kc in range(HC):
                        nc.tensor.matmul(pys[vh][:], ctxT[:, kc, :],
                                         WoT[:, kc, vh * VH:(vh + 1) * VH],
                                         start=(kc == 0), stop=False)
                    nc.tensor.matmul(pys[vh][:], ones_row_bf[:, 0:P],
                                     b_bf["bo"][:, vh * VH:(vh + 1) * VH],
                                     start=False, stop=True)
                y = y_p.tile([P, h], f32)
                for vh in range(2):
                    nc.vector.tensor_tensor(y[:, vh * VH:(vh + 1) * VH],
                                            pys[vh][:],
                                            hidR[:, half, vh * VH:(vh + 1) * VH],
                                            Alu.add)
                mu = y_p.tile([P, 1], f32)
                nc.vector.tensor_reduce(mu[:], y[:], AxisX, Alu.add)
                nc.vector.tensor_scalar(mu[:], mu[:], 1.0 / h, None, Alu.mult)
                yc = y_p.tile([P, h], f32)
                nc.vector.tensor_scalar(yc[:], y[:], mu[:], None, Alu.subtract)
                ssq = y_p.tile([P, 1], f32)
                nc.scalar.activation(y[:], yc[:], Act.Square, accum_out=ssq[:])
                std = y_p.tile([P, 1], f32)
                nc.scalar.activation(std[:], ssq[:], Act.Sqrt,
                                     scale=1.0 / h, bias=eps_t[:])
                rstd = y_p.tile([P, 1], f32)
                nc.vector.reciprocal(rstd[:], std[:])
                o1 = y_p.tile([P, h], f32)
                nc.vector.tensor_scalar(o1[:], yc[:], rstd[:], None, Alu.mult)
                nc.vector.tensor_tensor(o1[:], o1[:], bcast["ln_gamma"][:], Alu.mult)
                nc.vector.tensor_tensor(o1[:], o1[:], bcast["ln_beta"][:], Alu.add)
                nc.sync.dma_start(d_out[half], o1[:])

    nc.compile()
    return nc


def _shard_inputs(inputs):
    hs = np.ascontiguousarray(np.asarray(inputs["hidden_states"]), dtype=np.float32)
    bpe = np.asarray(inputs["bbox_pos_emb"])
    ident = np.eye(P, dtype=np.float32)
    in_maps = []
    for c in range(N_CORES):
        b = c // 4
        q0 = (c % 4) * I_CORE
        m = {
            "hid_full": np.ascontiguousarray(hs[b].reshape(S // P, P, H)),
            "hid_rows": np.ascontiguousarray(
                hs[b, q0:q0 + I_CORE].reshape(I_CORE // P, P, H)),
            "bpe": np.ascontiguousarray(
                bpe[q0:q0 + I_CORE, :, b, :], dtype=np.float32).reshape(
                    I_CORE, S // P, P, DH),
            "ident": ident,
        }
        for w in ("Wq", "Wk", "Wv", "Wo"):
            m[w] = np.ascontiguousarray(
                np.asarray(inputs[w], dtype=np.float32).reshape(H // P, P, H))
        for bn in ("bq", "bk", "bv", "bo", "ln_gamma", "ln_beta"):
            m[bn] = np.ascontiguousarray(
                np.asarray(inputs[bn], dtype=np.float32).reshape(1, H))
        in_maps.append(m)
    return in_maps


def _install_ntff_shim():
    """The agent image's antenv lacks axon_hooks; recreate the NTFF profile
    hook via ctypes against libaxon_pjrt.so so trace=True yields
    exec_time_ns + a perfetto trace."""
    import sys as _sys
    if "antenv.axon_hooks" in _sys.modules:
        return
    import types, ctypes, contextlib
    so_path = "/opt/axon/libaxon_pjrt.so"
    mod = types.ModuleType("antenv.axon_hooks")
    _state = {}

    def get_axon_ntff_profile_hook():
        if "hook" in _state:
            return _state["hook"]
        try:
            lib = ctypes.CDLL(so_path)
            if not hasattr(lib, "axon_start_nrt_profile"):
                _state["hook"] = None
                return None
            lib.axon_start_nrt_profile.argtypes = [
                ctypes.POINTER(ctypes.c_int64), ctypes.c_size_t]
            lib.axon_start_nrt_profile.restype = ctypes.c_int64
            lib.axon_stop_nrt_profile.argtypes = [ctypes.c_char_p]
            lib.axon_stop_nrt_profile.restype = ctypes.c_int64
        except OSError:
            _state["hook"] = None
            return None

        @contextlib.contextmanager
        def _hook(output_dir, device_ids):
            import jax
            jax.devices()
            if device_ids:
                ids = (ctypes.c_int64 * len(device_ids))(*device_ids)
                rc = lib.axon_start_nrt_profile(ids, len(device_ids))
            else:
                rc = lib.axon_start_nrt_profile(None, 0)
            if rc != 0:
                raise RuntimeError(f"axon_start_nrt_profile rc={rc}")
            try:
                yield
            finally:
                n = lib.axon_stop_nrt_profile(str(output_dir).encode())
                print(f"ntff profile: {n} file(s) written to {output_dir}")

        _state["hook"] = _hook
        return _hook

    mod.get_axon_ntff_profile_hook = get_axon_ntff_profile_hook
    _sys.modules["antenv.axon_hooks"] = mod


def kernel(**inputs):
    from concourse.bass_utils import run_bass_kernel_spmd

    if os.environ.get("BASS_KERNEL_TRACE"):
        _install_ntff_shim()
        import concourse.bass_utils as _bu
        _bu.upload_artifacts = lambda tmpdir: f"file://{tmpdir}"

    if "nc" not in _COMPILED:
        _COMPILED["nc"] = build_kernel()
    nc = _COMPILED["nc"]
    in_maps = _shard_inputs(inputs)
    res = run_bass_kernel_spmd(nc, in_maps, core_ids=list(range(N_CORES)),
                               trace=bool(os.environ.get("BASS_KERNEL_TRACE")))
    _COMPILED["last_result"] = res
    out = np.zeros((B, S, H), dtype=np.float32)
    for c in range(N_CORES):
        b = c // 4
        q0 = (c % 4) * I_CORE
        out[b, q0:q0 + I_CORE] = np.asarray(
            res.results[c]["out"]).reshape(I_CORE, H)
    return out


# revision 24
# speedup vs baseline: 1.5740x; 1.2465x over previous
"""Distributed Trainium2 Bass kernel for BrosAttention.

B=2, S=1024, H=768, NH=12, DH=64:
  q,k,v = heads(hidden @ W.T + b)
  scores = q@k^T + einsum('bnid,bijd->bnij', q, bpe)   (bpe = bbox transposed)
  probs  = softmax(scores / 8)
  out    = LN(probs@v @ Wo.T + bo + hidden)

Sharding: 8 cores = 2 batches x 4 query-row blocks of 256 rows. Each core
reads only its 64MB slice of bbox_pos_emb, computes K/V for the full
sequence of its batch (duplicated 4x, cheaper than a collective here), and
writes a disjoint [256, 768] output slice. No collectives.

Layout: transposed scores (scoresT[j, i] per head) because the bias term
q.bpe needs d on partitions; bpe arrives [j, d] and is PE-transposed with
two query rows packed per [128, j] tile. The bias matmul (lhsT = q of one
row as a [64, 12] weight) runs 4 i's concurrently in the four 32-column
groups of the PE array; bias tiles are PE-transposed again into [j, (i,n)]
and added to QK^T psum tiles via a stride-12 AP. Softmax-over-partitions
uses ones-vector matmuls; probs stay unnormalized until after P@V.
"""

import os
import sys
import numpy as np

sys.path.insert(0, "/opt/trn_rl_repo")

B, S, H, NH, DH = 2, 1024, 768, 12, 64
EPS = 1e-12
P = 128
I_CORE = S * B // 8  # 256
N_CORES = 8

_COMPILED = {}


def build_kernel(s=S, i_core=I_CORE, h=H, nh=NH, dh=DH):
    from contextlib import ExitStack
    from concourse import bacc, bass, mybir, tile

    f32 = mybir.dt.float32
    bf16 = mybir.dt.bfloat16
    Alu = mybir.AluOpType
    Act = mybir.ActivationFunctionType
    AxisX = mybir.AxisListType.X

    SC = s // P          # 8 seq chunks
    HC = h // P          # 6 hidden chunks
    IH = i_core // 2     # 128 i's per half
    NDUO_H = IH // 4     # 32 duos per half
    JH = min(512, s)     # fp32 matmul N limit / psum bank
    NJH = s // JH        # 2
    HP = nh // 2         # 6 head pairs
    VH = h // 2          # 384

    nc = bacc.Bacc(None, target_bir_lowering=False, debug=False)

    d_hidF = nc.declare_dram_parameter("hid_full", [SC, P, h], f32, isOutput=False)
    d_hidR = nc.declare_dram_parameter("hid_rows", [i_core // P, P, h], f32, isOutput=False)
    d_bpe = nc.declare_dram_parameter("bpe", [i_core, SC, P, dh], f32, isOutput=False)
    d_W = {w: nc.declare_dram_parameter(w, [HC, P, h], f32, isOutput=False)
           for w in ("Wq", "Wk", "Wv", "Wo")}
    d_b = {bn: nc.declare_dram_parameter(bn, [1, h], f32, isOutput=False)
           for bn in ("bq", "bk", "bv", "bo", "ln_gamma", "ln_beta")}
    d_ident = nc.declare_dram_parameter("ident", [P, P], f32, isOutput=False)
    d_out = nc.declare_dram_parameter("out", [i_core // P, P, h], f32, isOutput=True)

    with tile.TileContext(nc) as tc, ExitStack() as ctx:
        # ------------- long-lived pools -------------
        const_p = ctx.enter_context(tc.tile_pool(name="const", bufs=1))
        stat_p = ctx.enter_context(tc.tile_pool(name="stat", bufs=1))
        ps128 = ctx.enter_context(
            tc.tile_pool(name="ps128", bufs=2, space=bass.MemorySpace.PSUM))
        ps512 = ctx.enter_context(
            tc.tile_pool(name="ps512", bufs=2, space=bass.MemorySpace.PSUM))
        psB = ctx.enter_context(
            tc.tile_pool(name="psB", bufs=2, space=bass.MemorySpace.PSUM))
        psS = ctx.enter_context(
            tc.tile_pool(name="psS", bufs=1, space=bass.MemorySpace.PSUM))
        psC = ctx.enter_context(
            tc.tile_pool(name="psC", bufs=1, space=bass.MemorySpace.PSUM))

        # ------------- constants -------------
        ident = const_p.tile([P, P], f32)
        nc.sync.dma_start(ident[:], d_ident[:])
        ones_col = const_p.tile([P, 1], f32)
        nc.vector.memset(ones_col[:], 1.0)
        ones_row = const_p.tile([1, s], f32)
        nc.vector.memset(ones_row[:], 1.0)
        eps_t = const_p.tile([P, 1], f32)
        nc.vector.memset(eps_t[:], EPS)
        zrow = const_p.tile([1, P], bf16)
        nc.vector.memset(zrow[:], 0.0)
        ident_bf = const_p.tile([P, P], bf16)
        nc.vector.tensor_copy(ident_bf[:], ident[:])
        ones_col_bf = const_p.tile([P, 1], bf16)
        nc.vector.memset(ones_col_bf[:], 1.0)
        ones_row_bf = const_p.tile([1, s], bf16)
        nc.vector.memset(ones_row_bf[:], 1.0)
        b_sb = {}
        b_bf = {}
        for bn in ("bq", "bk", "bv", "bo", "ln_gamma", "ln_beta"):
            b_sb[bn] = const_p.tile([1, h], f32, name=f"bias_{bn}")
            nc.sync.dma_start(b_sb[bn][:], d_b[bn][:])
            b_bf[bn] = const_p.tile([1, h], bf16, name=f"biasbf_{bn}")
            nc.vector.tensor_copy(b_bf[bn][:], b_sb[bn][:])

        bcast = {}
        for bn in ("ln_gamma", "ln_beta"):
            t = stat_p.tile([P, h], f32, name=f"bcast_{bn}")
            for c in range(HC):
                pbx = ps128.tile([P, P], f32, name="pt")
                nc.tensor.matmul(pbx[:], ones_row[:, 0:P],
                                 b_sb[bn][:, c * P:(c + 1) * P])
                nc.scalar.copy(t[:, c * P:(c + 1) * P], pbx[:])
            bcast[bn] = t

        # long-lived activations
        hidR = stat_p.tile([P, i_core // P, h], f32)
        nc.sync.dma_start(hidR[:], d_hidR[:].transpose([1, 0, 2]))
        WoT = stat_p.tile([P, HC, h], bf16)
        qT128 = stat_p.tile([P, nh, i_core], bf16)  # q[n,i,:] at both 64-halves
        kT128 = stat_p.tile([P, HP, s], bf16)
        v_sb = stat_p.tile([P, SC, h], bf16)

        def pe_T(dst_ap, src_ap, copy_eng):
            bf = src_ap.dtype == bf16
            pt = ps128.tile([P, P], bf16 if bf else f32, name="pt")
            n = src_ap.shape[-1]
            nc.tensor.transpose(pt[0:n, :], src_ap,
                                ident_bf[:] if bf else ident[:])
            if copy_eng is nc.scalar:
                copy_eng.copy(dst_ap, pt[0:n, :])
            else:
                copy_eng.tensor_copy(dst_ap, pt[0:n, :])

        # ------------- phase 0 -------------
        with tc.tile_pool(name="early", bufs=1) as early_p, \
             tc.tile_pool(name="wnat", bufs=1) as wnat_p, \
             tc.tile_pool(name="wtw", bufs=1) as wtw_p:
            hidF = early_p.tile([P, SC, h], f32)
            nc.sync.dma_start(hidF[:], d_hidF[:].transpose([1, 0, 2]))
            hidT = early_p.tile([P, HC, s], bf16)
            for rc in range(HC):
                for jc in range(SC):
                    pe_T(hidT[:, rc, jc * P:(jc + 1) * P],
                         hidF[:, jc, rc * P:(rc + 1) * P], nc.vector)
            hidRT = early_p.tile([P, HC, i_core], bf16)
            for rc in range(HC):
                for ic in range(i_core // P):
                    pe_T(hidRT[:, rc, ic * P:(ic + 1) * P],
                         hidR[:, ic, rc * P:(rc + 1) * P], nc.vector)

            def load_WT(w, dst):
                wn = wnat_p.tile([P, HC, h], f32, name="wnat")
                nc.sync.dma_start(wn[:], d_W[w][:].transpose([1, 0, 2]))
                for rc in range(HC):
                    for c in range(HC):
                        pe_T(dst[:, rc, c * P:(c + 1) * P],
                             wn[:, c, rc * P:(rc + 1) * P], nc.scalar)
                return dst

            # Q projection (transposed): qT = Wq @ hidR^T + bq
            WqT = load_WT("Wq", wtw_p.tile([P, HC, h], bf16, name="WT"))
            for r in range(HC):
                pq = ps512.tile([P, JH], f32, name="big")
                for kc in range(HC):
                    nc.tensor.matmul(pq[:, 0:i_core],
                                     WqT[:, kc, r * P:(r + 1) * P],
                                     hidRT[:, kc, :], start=(kc == 0), stop=False)
                nc.tensor.matmul(pq[:, 0:i_core], b_bf["bq"][:, r * P:(r + 1) * P],
                                 ones_row_bf[:, 0:i_core], start=False, stop=True)
                for sub in range(2):
                    src = pq[sub * dh:(sub + 1) * dh, 0:i_core]
                    nc.vector.tensor_copy(qT128[0:dh, 2 * r + sub, :], src)
                    nc.vector.tensor_copy(qT128[dh:P, 2 * r + sub, :], src)

            # K projection (transposed): kT = Wk @ hid^T + bk
            WkT = load_WT("Wk", wtw_p.tile([P, HC, h], bf16, name="WT"))
            for r in range(HC):
                for jh in range(NJH):
                    pk = ps512.tile([P, JH], f32, name="big")
                    for kc in range(HC):
                        nc.tensor.matmul(pk[:], WkT[:, kc, r * P:(r + 1) * P],
                                         hidT[:, kc, jh * JH:(jh + 1) * JH],
                                         start=(kc == 0), stop=False)
                    nc.tensor.matmul(pk[:], b_bf["bk"][:, r * P:(r + 1) * P],
                                     ones_row_bf[:, 0:JH], start=False, stop=True)
                    nc.vector.tensor_copy(
                        kT128[:, r, jh * JH:(jh + 1) * JH], pk[:])

            # V projection (natural): v = hid @ Wv^T + bv
            WvT = load_WT("Wv", wtw_p.tile([P, HC, h], bf16, name="WT"))
            for jc in range(SC):
                for vh in range(2):
                    pv = ps512.tile([P, JH], f32, name="big")
                    for kc in range(HC):
                        nc.tensor.matmul(pv[:, 0:VH],
                                         hidT[:, kc, jc * P:(jc + 1) * P],
                                         WvT[:, kc, vh * VH:(vh + 1) * VH],
                                         start=(kc == 0), stop=False)
                    nc.tensor.matmul(pv[:, 0:VH], ones_row_bf[:, 0:P],
                                     b_bf["bv"][:, vh * VH:(vh + 1) * VH],
                                     start=False, stop=True)
                    nc.vector.tensor_copy(v_sb[:, jc, vh * VH:(vh + 1) * VH],
                                          pv[:, 0:VH])

            load_WT("Wo", WoT)

        # ------------- phases A+B -------------
        with tc.tile_pool(name="bpeN", bufs=2) as bpeN_p, \
             tc.tile_pool(name="bpeT", bufs=2) as bpeT_p, \
             tc.tile_pool(name="bias4", bufs=1) as bias4_p, \
             tc.tile_pool(name="biasT", bufs=1) as biasT_p, \
             tc.tile_pool(name="sm", bufs=2) as sm_p, \
             tc.tile_pool(name="ctxp", bufs=1) as ctx_p, \
             tc.tile_pool(name="yp", bufs=1) as y_p:
            for half in range(2):
                i0h = half * IH
                # biasT[j, jc, duo*48 + 12*i4 + n]
                biasT = biasT_p.tile([P, SC, NDUO_H * 48], bf16)

                for duo in range(NDUO_H):
                    pb_h = [psB.tile([P, JH], f32, name="pbh") for j in range(NJH)]
                    for jh in range(NJH):
                        nc.tensor.matmul(pb_h[jh][:], zrow[:],
                                         ones_row_bf[:, 0:JH],
                                         start=True, stop=False,
                                         skip_group_check=True)
                    for pp in range(2):
                        iA = i0h + 4 * duo + 2 * pp
                        bpeT = bpeT_p.tile([P, SC, P], bf16)
                        bn2 = bpeN_p.tile([P, SC, 2, dh], f32)
                        for ii in range(2):
                            nc.sync.dma_start(bn2[:, :, ii, :],
                                              d_bpe[iA + ii].transpose([1, 0, 2]))
                        # one [128,128] transpose per jc packs (i0-d | i1-d)
                        NJC = JH // P
                        for jg in range(NJH):
                            pt4 = ps512.tile([P, JH], f32, name="big")
                            for jl in range(NJC):
                                jc = jg * NJC + jl
                                nc.tensor.transpose(
                                    pt4[:, jl * P:(jl + 1) * P],
                                    bn2[:, jc, :, :], ident[:])
                            dst = bpeT[:, jg * NJC:(jg + 1) * NJC, :]
                            srcv = pt4[:].rearrange("p (a b) -> p a b", a=NJC)
                            if (pp * NJH + jg) % 2 == 0:
                                nc.scalar.copy(dst, srcv)
                            else:
                                nc.vector.tensor_copy(dst, srcv)
                        # bias matmuls: 4 i's of the duo -> 4 col groups
                        for ii in range(2):
                            cg = 32 * (2 * pp + ii)
                            i_loc = iA + ii
                            lhs = qT128[ii * dh:(ii + 1) * dh, :,
                                        i_loc:i_loc + 1]
                            for jh in range(NJH):
                                jc0 = (jh * JH) // P
                                rhs = bpeT[ii * dh:(ii + 1) * dh,
                                           jc0:jc0 + JH // P, :]
                                nc.tensor.matmul(
                                    pb_h[jh][cg:cg + nh, :], lhs, rhs,
                                    start=False, stop=(pp == 1 and ii == 1),
                                    skip_group_check=True,
                                    tile_position=(ii * dh, cg))
                    b4 = bias4_p.tile([P, s], bf16)
                    for jh in range(NJH):
                        nc.vector.tensor_copy(b4[:, jh * JH:(jh + 1) * JH],
                                              pb_h[jh][:])
                    for jc in range(SC):
                        ptb = ps128.tile([P, P], bf16, name="pt")
                        nc.tensor.transpose(ptb[:], b4[:, jc * P:(jc + 1) * P],
                                            ident_bf[:])
                        nc.vector.tensor_copy(
                            biasT[:, jc, duo * 48:(duo + 1) * 48].rearrange(
                                "p (a b) -> p a b", a=4),
                            ptb[:].rearrange("p (a b) -> p a b", a=4)[:, :, 0:12])

                # ---- attention ----
                ctxT = ctx_p.tile([P, HP, IH], bf16)
                for hp in range(HP):
                    pctx = psC.tile([P, IH], f32, name="pctx")
                    for sub in range(2):
                        n = 2 * hp + sub
                        probsT = sm_p.tile([P, SC, IH], bf16)
                        psum_s = psS.tile([1, IH], f32)
                        for jc in range(SC):
                            pqk = ps128.tile([P, IH], f32, name="pt")
                            sb = sub * dh
                            nc.tensor.matmul(pqk[:],
                                             kT128[sb:sb + dh, hp, jc * P:(jc + 1) * P],
                                             qT128[sb:sb + dh, n, i0h:i0h + IH])
                            sE = sm_p.tile([P, IH], f32)
                            nc.vector.tensor_tensor(
                                sE[:], pqk[:],
                                biasT[:, jc, n:n + 12 * (IH - 1) + 1:12], Alu.add)
                            nc.scalar.activation(probsT[:, jc, :], sE[:],
                                                 Act.Exp, scale=0.125)
                            nc.tensor.matmul(psum_s[:], ones_col_bf[:],
                                             probsT[:, jc, :],
                                             start=(jc == 0), stop=(jc == SC - 1),
                                             skip_group_check=True)
                        rec = sm_p.tile([1, IH], f32)
                        nc.vector.reciprocal(rec[:], psum_s[:])
                        prec = ps128.tile([P, IH], f32, name="pt")
                        nc.tensor.matmul(prec[0:dh, :], ones_row[:, 0:dh], rec[:])
                        recB = sm_p.tile([dh, IH], f32)
                        nc.scalar.copy(recB[:], prec[0:dh, :])
                        for jc in range(SC):
                            nc.tensor.matmul(
                                pctx[sub * dh:(sub + 1) * dh, :],
                                v_sb[:, jc, n * dh:(n + 1) * dh],
                                probsT[:, jc, :],
                                start=(jc == 0), stop=(jc == SC - 1),
                                tile_position=(0, sub * dh),
                                skip_group_check=True)
                        nc.vector.tensor_tensor(
                            pctx[sub * dh:(sub + 1) * dh, :],
                            pctx[sub * dh:(sub + 1) * dh, :],
                            recB[:], Alu.mult)
                    nc.scalar.copy(ctxT[:, hp, :], pctx[:])

                # ---- O-proj + residual + LN ----
                pys = [ps512.tile([P, VH], f32, name="big") for j in range(2)]
                for vh in range(2):
                    for kc in range(HC):
                        nc.tensor.matmul(pys[vh][:], ctxT[:, kc, :],
                                         WoT[:, kc, vh * VH:(vh + 1) * VH],
                                         start=(kc == 0), stop=False)
                    nc.tensor.matmul(pys[vh][:], ones_row_bf[:, 0:P],
                                     b_bf["bo"][:, vh * VH:(vh + 1) * VH],
                                     start=False, stop=True)
                y = y_p.tile([P, h], f32)
                for vh in range(2):
                    nc.vector.tensor_tensor(y[:, vh * VH:(vh + 1) * VH],
                                            pys[vh][:],
                                            hidR[:, half, vh * VH:(vh + 1) * VH],
                                            Alu.add)
                mu = y_p.tile([P, 1], f32)
                nc.vector.tensor_reduce(mu[:], y[:], AxisX, Alu.add)
                nc.vector.tensor_scalar(mu[:], mu[:], 1.0 / h, None, Alu.mult)
                yc = y_p.tile([P, h], f32)
                nc.vector.tensor_scalar(yc[:], y[:], mu[:], None, Alu.subtract)
                ssq = y_p.tile([P, 1], f32)
                nc.scalar.activation(y[:], yc[:], Act.Square, accum_out=ssq[:])
                std = y_p.tile([P, 1], f32)
                nc.scalar.activation(std[:], ssq[:], Act.Sqrt,
                                     scale=1.0 / h, bias=eps_t[:])
                rstd = y_p.tile([P, 1], f32)
                nc.vector.reciprocal(rstd[:], std[:])
                o1 = y_p.tile([P, h], f32)
                nc.vector.tensor_scalar(o1[:], yc[:], rstd[:], None, Alu.mult)
                nc.vector.tensor_tensor(o1[:], o1[:], bcast["ln_gamma"][:], Alu.mult)
                nc.vector.tensor_tensor(o1[:], o1[:], bcast["ln_beta"][:], Alu.add)
                nc.sync.dma_start(d_out[half], o1[:])

    nc.compile()
    return nc


def _shard_inputs(inputs):
    hs = np.ascontiguousarray(np.asarray(inputs["hidden_states"]), dtype=np.float32)
    bpe = np.asarray(inputs["bbox_pos_emb"])
    ident = np.eye(P, dtype=np.float32)
    in_maps = []
    for c in range(N_CORES):
        b = c // 4
        q0 = (c % 4) * I_CORE
        m = {
            "hid_full": np.ascontiguousarray(hs[b].reshape(S // P, P, H)),
            "hid_rows": np.ascontiguousarray(
                hs[b, q0:q0 + I_CORE].reshape(I_CORE // P, P, H)),
            "bpe": np.ascontiguousarray(
                bpe[q0:q0 + I_CORE, :, b, :], dtype=np.float32).reshape(
                    I_CORE, S // P, P, DH),
            "ident": ident,
        }
        for w in ("Wq", "Wk", "Wv", "Wo"):
            m[w] = np.ascontiguousarray(
                np.asarray(inputs[w], dtype=np.float32).reshape(H // P, P, H))
        for bn in ("bq", "bk", "bv", "bo", "ln_gamma", "ln_beta"):
            m[bn] = np.ascontiguousarray(
                np.asarray(inputs[bn], dtype=np.float32).reshape(1, H))
        in_maps.append(m)
    return in_maps


def _install_ntff_shim():
    """The agent image's antenv lacks axon_hooks; recreate the NTFF profile
    hook via ctypes against libaxon_pjrt.so so trace=True yields
    exec_time_ns + a perfetto trace."""
    import sys as _sys
    if "antenv.axon_hooks" in _sys.modules:
        return
    import types, ctypes, contextlib
    so_path = "/opt/axon/libaxon_pjrt.so"
    mod = types.ModuleType("antenv.axon_hooks")
    _state = {}

    def get_axon_ntff_profile_hook():
        if "hook" in _state:
            return _state["hook"]
        try:
            lib = ctypes.CDLL(so_path)
            if not hasattr(lib, "axon_start_nrt_profile"):
                _state["hook"] = None
                return None
            lib.axon_start_nrt_profile.argtypes = [
                ctypes.POINTER(ctypes.c_int64), ctypes.c_size_t]
            lib.axon_start_nrt_profile.restype = ctypes.c_int64
            lib.axon_stop_nrt_profile.argtypes = [ctypes.c_char_p]
            lib.axon_stop_nrt_profile.restype = ctypes.c_int64
        except OSError:
            _state["hook"] = None
            return None

        @contextlib.contextmanager
        def _hook(output_dir, device_ids):
            import jax
            jax.devices()
            if device_ids:
                ids = (ctypes.c_int64 * len(device_ids))(*device_ids)
                rc = lib.axon_start_nrt_profile(ids, len(device_ids))
            else:
                rc = lib.axon_start_nrt_profile(None, 0)
            if rc != 0:
                raise RuntimeError(f"axon_start_nrt_profile rc={rc}")
            try:
                yield
            finally:
                n = lib.axon_stop_nrt_profile(str(output_dir).encode())
                print(f"ntff profile: {n} file(s) written to {output_dir}")

        _state["hook"] = _hook
        return _hook

    mod.get_axon_ntff_profile_hook = get_axon_ntff_profile_hook
    _sys.modules["antenv.axon_hooks"] = mod


def kernel(**inputs):
    from concourse.bass_utils import run_bass_kernel_spmd

    if os.environ.get("BASS_KERNEL_TRACE"):
        _install_ntff_shim()
        import concourse.bass_utils as _bu
        _bu.upload_artifacts = lambda tmpdir: f"file://{tmpdir}"

    if "nc" not in _COMPILED:
        _COMPILED["nc"] = build_kernel()
    nc = _COMPILED["nc"]
    in_maps = _shard_inputs(inputs)
    res = run_bass_kernel_spmd(nc, in_maps, core_ids=list(range(N_CORES)),
                               trace=bool(os.environ.get("BASS_KERNEL_TRACE")))
    _COMPILED["last_result"] = res
    out = np.zeros((B, S, H), dtype=np.float32)
    for c in range(N_CORES):
        b = c // 4
        q0 = (c % 4) * I_CORE
        out[b, q0:q0 + I_CORE] = np.asarray(
            res.results[c]["out"]).reshape(I_CORE, H)
    return out


# revision 26
# speedup vs baseline: 1.9979x; 1.2693x over previous
"""Distributed Trainium2 Bass kernel for BrosAttention.

B=2, S=1024, H=768, NH=12, DH=64:
  q,k,v = heads(hidden @ W.T + b)
  scores = q@k^T + einsum('bnid,bijd->bnij', q, bpe)   (bpe = bbox transposed)
  probs  = softmax(scores / 8)
  out    = LN(probs@v @ Wo.T + bo + hidden)

Sharding: 8 cores = 2 batches x 4 query-row blocks of 256 rows. Each core
reads only its 64MB slice of bbox_pos_emb, computes K/V for the full
sequence of its batch (duplicated 4x, cheaper than a collective here), and
writes a disjoint [256, 768] output slice. No collectives.

Layout: transposed scores (scoresT[j, i] per head) because the bias term
q.bpe needs d on partitions; bpe arrives [j, d] and is PE-transposed with
two query rows packed per [128, j] tile. The bias matmul (lhsT = q of one
row as a [64, 12] weight) runs 4 i's concurrently in the four 32-column
groups of the PE array; bias tiles are PE-transposed again into [j, (i,n)]
and added to QK^T psum tiles via a stride-12 AP. Softmax-over-partitions
uses ones-vector matmuls; probs stay unnormalized until after P@V.
"""

import os
import sys
import numpy as np

sys.path.insert(0, "/opt/trn_rl_repo")

B, S, H, NH, DH = 2, 1024, 768, 12, 64
EPS = 1e-12
P = 128
I_CORE = S * B // 8  # 256
N_CORES = 8

_COMPILED = {}


def build_kernel(s=S, i_core=I_CORE, h=H, nh=NH, dh=DH):
    from contextlib import ExitStack
    from concourse import bacc, bass, mybir, tile

    f32 = mybir.dt.float32
    bf16 = mybir.dt.bfloat16
    Alu = mybir.AluOpType
    Act = mybir.ActivationFunctionType
    AxisX = mybir.AxisListType.X

    SC = s // P          # 8 seq chunks
    HC = h // P          # 6 hidden chunks
    IH = i_core // 2     # 128 i's per half
    NDUO_H = IH // 4     # 32 duos per half
    JH = min(512, s)     # fp32 matmul N limit / psum bank
    NJH = s // JH        # 2
    HP = nh // 2         # 6 head pairs
    VH = h // 2          # 384

    nc = bacc.Bacc(None, target_bir_lowering=False, debug=False)

    bf16_ = mybir.dt.bfloat16
    d_hidT = nc.declare_dram_parameter("hidT", [HC, P, s], bf16_, isOutput=False)
    d_hidRT = nc.declare_dram_parameter("hidRT", [HC, P, i_core], bf16_, isOutput=False)
    d_hidR = nc.declare_dram_parameter("hid_rows", [i_core // P, P, h], f32, isOutput=False)
    d_bpe = nc.declare_dram_parameter("bpe", [i_core, dh, s], bf16_, isOutput=False)
    d_W = {w: nc.declare_dram_parameter(w + "T", [HC, P, h], bf16_, isOutput=False)
           for w in ("Wq", "Wk", "Wv", "Wo")}
    d_b = {bn: nc.declare_dram_parameter(bn, [1, h], f32, isOutput=False)
           for bn in ("bq", "bk", "bv", "bo", "ln_gamma", "ln_beta")}
    d_ident = nc.declare_dram_parameter("ident", [P, P], f32, isOutput=False)
    d_out = nc.declare_dram_parameter("out", [i_core // P, P, h], f32, isOutput=True)

    with tile.TileContext(nc) as tc, ExitStack() as ctx:
        # ------------- long-lived pools -------------
        const_p = ctx.enter_context(tc.tile_pool(name="const", bufs=1))
        stat_p = ctx.enter_context(tc.tile_pool(name="stat", bufs=1))
        ps128 = ctx.enter_context(
            tc.tile_pool(name="ps128", bufs=3, space=bass.MemorySpace.PSUM))
        ps512 = ctx.enter_context(
            tc.tile_pool(name="ps512", bufs=1, space=bass.MemorySpace.PSUM))
        psB = ctx.enter_context(
            tc.tile_pool(name="psB", bufs=2, space=bass.MemorySpace.PSUM))
        psS = ctx.enter_context(
            tc.tile_pool(name="psS", bufs=1, space=bass.MemorySpace.PSUM))
        psC = ctx.enter_context(
            tc.tile_pool(name="psC", bufs=1, space=bass.MemorySpace.PSUM))

        # ------------- constants -------------
        ident = const_p.tile([P, P], f32)
        nc.sync.dma_start(ident[:], d_ident[:])
        ones_col = const_p.tile([P, 1], f32)
        nc.vector.memset(ones_col[:], 1.0)
        ones_row = const_p.tile([1, s], f32)
        nc.vector.memset(ones_row[:], 1.0)
        eps_t = const_p.tile([P, 1], f32)
        nc.vector.memset(eps_t[:], EPS)
        zrow = const_p.tile([1, P], bf16)
        nc.vector.memset(zrow[:], 0.0)
        ident_bf = const_p.tile([P, P], bf16)
        nc.vector.tensor_copy(ident_bf[:], ident[:])
        ones_col_bf = const_p.tile([P, 1], bf16)
        nc.vector.memset(ones_col_bf[:], 1.0)
        ones_row_bf = const_p.tile([1, s], bf16)
        nc.vector.memset(ones_row_bf[:], 1.0)
        b_sb = {}
        b_bf = {}
        for bn in ("bq", "bk", "bv", "bo", "ln_gamma", "ln_beta"):
            b_sb[bn] = const_p.tile([1, h], f32, name=f"bias_{bn}")
            nc.sync.dma_start(b_sb[bn][:], d_b[bn][:])
            b_bf[bn] = const_p.tile([1, h], bf16, name=f"biasbf_{bn}")
            nc.vector.tensor_copy(b_bf[bn][:], b_sb[bn][:])

        bcast = {}
        for bn in ("ln_gamma", "ln_beta"):
            t = stat_p.tile([P, h], f32, name=f"bcast_{bn}")
            for c in range(HC):
                pbx = ps128.tile([P, P], f32, name="pt")
                nc.tensor.matmul(pbx[:], ones_row[:, 0:P],
                                 b_sb[bn][:, c * P:(c + 1) * P])
                nc.scalar.copy(t[:, c * P:(c + 1) * P], pbx[:])
            bcast[bn] = t

        # long-lived activations
        hidR = stat_p.tile([P, i_core // P, h], f32)
        nc.sync.dma_start(hidR[:], d_hidR[:].transpose([1, 0, 2]))
        WoT = stat_p.tile([P, HC, h], bf16)
        nc.sync.dma_start(WoT[:], d_W["Wo"][:].transpose([1, 0, 2]))
        qT128 = stat_p.tile([P, nh, i_core], bf16)  # q[n,i,:] at both 64-halves
        kT128 = stat_p.tile([P, HP, s], bf16)
        v_sb = stat_p.tile([P, SC, h], bf16)

        def pe_T(dst_ap, src_ap, copy_eng):
            bf = src_ap.dtype == bf16
            pt = ps128.tile([P, P], bf16 if bf else f32, name="pt")
            n = src_ap.shape[-1]
            nc.tensor.transpose(pt[0:n, :], src_ap,
                                ident_bf[:] if bf else ident[:])
            if copy_eng is nc.scalar:
                copy_eng.copy(dst_ap, pt[0:n, :])
            else:
                copy_eng.tensor_copy(dst_ap, pt[0:n, :])

        # ------------- phase 0 -------------
        with tc.tile_pool(name="early", bufs=1) as early_p:
            hidT = early_p.tile([P, HC, s], bf16)
            nc.sync.dma_start(hidT[:], d_hidT[:].transpose([1, 0, 2]))
            hidRT = early_p.tile([P, HC, i_core], bf16)
            nc.sync.dma_start(hidRT[:], d_hidRT[:].transpose([1, 0, 2]))

            def load_WT(w, dst):
                nc.sync.dma_start(dst[:], d_W[w][:].transpose([1, 0, 2]))
                return dst

            # Q projection (transposed): qT = Wq @ hidR^T + bq
            WqT = load_WT("Wq", early_p.tile([P, HC, h], bf16, name="WT_q"))
            for r in range(HC):
                pq = ps512.tile([P, JH], f32, name="big")
                for kc in range(HC):
                    nc.tensor.matmul(pq[:, 0:i_core],
                                     WqT[:, kc, r * P:(r + 1) * P],
                                     hidRT[:, kc, :], start=(kc == 0), stop=False)
                nc.tensor.matmul(pq[:, 0:i_core], b_bf["bq"][:, r * P:(r + 1) * P],
                                 ones_row_bf[:, 0:i_core], start=False, stop=True)
                for sub in range(2):
                    src = pq[sub * dh:(sub + 1) * dh, 0:i_core]
                    nc.vector.tensor_copy(qT128[0:dh, 2 * r + sub, :], src)
                    nc.vector.tensor_copy(qT128[dh:P, 2 * r + sub, :], src)

            # K projection (transposed): kT = Wk @ hid^T + bk
            WkT = load_WT("Wk", early_p.tile([P, HC, h], bf16, name="WT_k"))
            for r in range(HC):
                for jh in range(NJH):
                    pk = ps512.tile([P, JH], f32, name="big")
                    for kc in range(HC):
                        nc.tensor.matmul(pk[:], WkT[:, kc, r * P:(r + 1) * P],
                                         hidT[:, kc, jh * JH:(jh + 1) * JH],
                                         start=(kc == 0), stop=False)
                    nc.tensor.matmul(pk[:], b_bf["bk"][:, r * P:(r + 1) * P],
                                     ones_row_bf[:, 0:JH], start=False, stop=True)
                    nc.vector.tensor_copy(
                        kT128[:, r, jh * JH:(jh + 1) * JH], pk[:])

            # V projection (natural): v = hid @ Wv^T + bv
            WvT = load_WT("Wv", early_p.tile([P, HC, h], bf16, name="WT_v"))
            for jc in range(SC):
                for vh in range(2):
                    pv = ps512.tile([P, JH], f32, name="big")
                    for kc in range(HC):
                        nc.tensor.matmul(pv[:, 0:VH],
                                         hidT[:, kc, jc * P:(jc + 1) * P],
                                         WvT[:, kc, vh * VH:(vh + 1) * VH],
                                         start=(kc == 0), stop=False)
                    nc.tensor.matmul(pv[:, 0:VH], ones_row_bf[:, 0:P],
                                     b_bf["bv"][:, vh * VH:(vh + 1) * VH],
                                     start=False, stop=True)
                    nc.vector.tensor_copy(v_sb[:, jc, vh * VH:(vh + 1) * VH],
                                          pv[:, 0:VH])


        # ------------- phases A+B -------------
        with tc.tile_pool(name="bpeT", bufs=4) as bpeT_p, \
             tc.tile_pool(name="bias4", bufs=1) as bias4_p, \
             tc.tile_pool(name="biasT", bufs=1) as biasT_p, \
             tc.tile_pool(name="sm", bufs=2) as sm_p, \
             tc.tile_pool(name="ctxp", bufs=1) as ctx_p, \
             tc.tile_pool(name="yp", bufs=1) as y_p:
            for half in range(2):
                i0h = half * IH
                # biasT[j, jc, duo*48 + 12*i4 + n]
                biasT = biasT_p.tile([P, SC, NDUO_H * 48], bf16)

                for duo in range(NDUO_H):
                    pb_h = [psB.tile([P, JH], f32, name="pbh") for j in range(NJH)]
                    for jh in range(NJH):
                        nc.tensor.matmul(pb_h[jh][:], zrow[:],
                                         ones_row_bf[:, 0:JH],
                                         start=True, stop=False,
                                         skip_group_check=True)
                    for pp in range(2):
                        iA = i0h + 4 * duo + 2 * pp
                        bpeT = bpeT_p.tile([P, s], bf16)
                        for ii in range(2):
                            nc.sync.dma_start(bpeT[ii * dh:(ii + 1) * dh, :],
                                              d_bpe[iA + ii])
                        # bias matmuls: 4 i's of the duo -> 4 col groups
                        for ii in range(2):
                            cg = 32 * (2 * pp + ii)
                            i_loc = iA + ii
                            lhs = qT128[ii * dh:(ii + 1) * dh, :,
                                        i_loc:i_loc + 1]
                            for jh in range(NJH):
                                rhs = bpeT[ii * dh:(ii + 1) * dh,
                                           jh * JH:(jh + 1) * JH]
                                nc.tensor.matmul(
                                    pb_h[jh][cg:cg + nh, :], lhs, rhs,
                                    start=False, stop=(pp == 1 and ii == 1),
                                    skip_group_check=True,
                                    tile_position=(ii * dh, cg))
                    b4 = bias4_p.tile([P, s], bf16)
                    for jh in range(NJH):
                        nc.vector.tensor_copy(b4[:, jh * JH:(jh + 1) * JH],
                                              pb_h[jh][:])
                    for jc in range(SC):
                        ptb = ps128.tile([P, P], bf16, name="pt")
                        nc.tensor.transpose(ptb[:], b4[:, jc * P:(jc + 1) * P],
                                            ident_bf[:])
                        nc.vector.tensor_copy(
                            biasT[:, jc, duo * 48:(duo + 1) * 48].rearrange(
                                "p (a b) -> p a b", a=4),
                            ptb[:].rearrange("p (a b) -> p a b", a=4)[:, :, 0:12])

                # ---- attention ----
                ctxT = ctx_p.tile([P, HP, IH], bf16)
                for hp in range(HP):
                    pctx = psC.tile([P, IH], f32, name="pctx")
                    for sub in range(2):
                        n = 2 * hp + sub
                        probsT = sm_p.tile([P, SC, IH], bf16)
                        psum_s = psS.tile([1, IH], f32)
                        for jc in range(SC):
                            pqk = ps128.tile([P, IH], f32, name="pt")
                            sb = sub * dh
                            nc.tensor.matmul(pqk[:],
                                             kT128[sb:sb + dh, hp, jc * P:(jc + 1) * P],
                                             qT128[sb:sb + dh, n, i0h:i0h + IH])
                            sE = sm_p.tile([P, IH], f32)
                            nc.vector.tensor_tensor(
                                sE[:], pqk[:],
                                biasT[:, jc, n:n + 12 * (IH - 1) + 1:12], Alu.add)
                            nc.scalar.activation(probsT[:, jc, :], sE[:],
                                                 Act.Exp, scale=0.125)
                            nc.tensor.matmul(psum_s[:], ones_col_bf[:],
                                             probsT[:, jc, :],
                                             start=(jc == 0), stop=(jc == SC - 1),
                                             skip_group_check=True)
                        rec = sm_p.tile([1, IH], f32)
                        nc.vector.reciprocal(rec[:], psum_s[:])
                        prec = ps128.tile([P, IH], f32, name="pt")
                        nc.tensor.matmul(prec[0:dh, :], ones_row[:, 0:dh], rec[:])
                        recB = sm_p.tile([dh, IH], f32)
                        nc.scalar.copy(recB[:], prec[0:dh, :])
                        for jc in range(SC):
                            nc.tensor.matmul(
                                pctx[sub * dh:(sub + 1) * dh, :],
                                v_sb[:, jc, n * dh:(n + 1) * dh],
                                probsT[:, jc, :],
                                start=(jc == 0), stop=(jc == SC - 1),
                                tile_position=(0, sub * dh),
                                skip_group_check=True)
                        nc.vector.tensor_tensor(
                            pctx[sub * dh:(sub + 1) * dh, :],
                            pctx[sub * dh:(sub + 1) * dh, :],
                            recB[:], Alu.mult)
                    nc.scalar.copy(ctxT[:, hp, :], pctx[:])

                # ---- O-proj + residual + LN ----
                pys = [ps512.tile([P, VH], f32, name="big") for j in range(2)]
                for vh in range(2):
                    for kc in range(HC):
                        nc.tensor.matmul(pys[vh][:], ctxT[:, kc, :],
                                         WoT[:, kc, vh * VH:(vh + 1) * VH],
                                         start=(kc == 0), stop=False)
                    nc.tensor.matmul(pys[vh][:], ones_row_bf[:, 0:P],
                                     b_bf["bo"][:, vh * VH:(vh + 1) * VH],
                                     start=False, stop=True)
                y = y_p.tile([P, h], f32)
                for vh in range(2):
                    nc.vector.tensor_tensor(y[:, vh * VH:(vh + 1) * VH],
                                            pys[vh][:],
                                            hidR[:, half, vh * VH:(vh + 1) * VH],
                                            Alu.add)
                mu = y_p.tile([P, 1], f32)
                nc.vector.tensor_reduce(mu[:], y[:], AxisX, Alu.add)
                nc.vector.tensor_scalar(mu[:], mu[:], 1.0 / h, None, Alu.mult)
                yc = y_p.tile([P, h], f32)
                nc.vector.tensor_scalar(yc[:], y[:], mu[:], None, Alu.subtract)
                ssq = y_p.tile([P, 1], f32)
                nc.scalar.activation(y[:], yc[:], Act.Square, accum_out=ssq[:])
                std = y_p.tile([P, 1], f32)
                nc.scalar.activation(std[:], ssq[:], Act.Sqrt,
                                     scale=1.0 / h, bias=eps_t[:])
                rstd = y_p.tile([P, 1], f32)
                nc.vector.reciprocal(rstd[:], std[:])
                o1 = y_p.tile([P, h], f32)
                nc.vector.tensor_scalar(o1[:], yc[:], rstd[:], None, Alu.mult)
                nc.vector.tensor_tensor(o1[:], o1[:], bcast["ln_gamma"][:], Alu.mult)
                nc.vector.tensor_tensor(o1[:], o1[:], bcast["ln_beta"][:], Alu.add)
                nc.sync.dma_start(d_out[half], o1[:])

    nc.compile()
    return nc


def _shard_inputs(inputs):
    import ml_dtypes
    bf = ml_dtypes.bfloat16
    hs = np.ascontiguousarray(np.asarray(inputs["hidden_states"]), dtype=np.float32)
    bpe = np.asarray(inputs["bbox_pos_emb"])
    ident = np.eye(P, dtype=np.float32)
    # per-batch transposed hidden [H, S] in bf16
    hsT = {b: np.ascontiguousarray(hs[b].T.astype(bf)).reshape(H // P, P, S)
           for b in range(B)}
    WT = {w: np.ascontiguousarray(
             np.asarray(inputs[w], dtype=np.float32).T.astype(bf)).reshape(
                 H // P, P, H)
          for w in ("Wq", "Wk", "Wv", "Wo")}
    in_maps = []
    for c in range(N_CORES):
        b = c // 4
        q0 = (c % 4) * I_CORE
        m = {
            "hidT": hsT[b],
            "hidRT": np.ascontiguousarray(
                hs[b, q0:q0 + I_CORE].T.astype(bf)).reshape(H // P, P, I_CORE),
            "hid_rows": np.ascontiguousarray(
                hs[b, q0:q0 + I_CORE].reshape(I_CORE // P, P, H)),
            "bpe": np.ascontiguousarray(
                bpe[q0:q0 + I_CORE, :, b, :].transpose(0, 2, 1).astype(bf)),
            "ident": ident,
        }
        for w in ("Wq", "Wk", "Wv", "Wo"):
            m[w + "T"] = WT[w]
        for bn in ("bq", "bk", "bv", "bo", "ln_gamma", "ln_beta"):
            m[bn] = np.ascontiguousarray(
                np.asarray(inputs[bn], dtype=np.float32).reshape(1, H))
        in_maps.append(m)
    return in_maps


def _install_ntff_shim():
    """The agent image's antenv lacks axon_hooks; recreate the NTFF profile
    hook via ctypes against libaxon_pjrt.so so trace=True yields
    exec_time_ns + a perfetto trace."""
    import sys as _sys
    if "antenv.axon_hooks" in _sys.modules:
        return
    import types, ctypes, contextlib
    so_path = "/opt/axon/libaxon_pjrt.so"
    mod = types.ModuleType("antenv.axon_hooks")
    _state = {}

    def get_axon_ntff_profile_hook():
        if "hook" in _state:
            return _state["hook"]
        try:
            lib = ctypes.CDLL(so_path)
            if not hasattr(lib, "axon_start_nrt_profile"):
                _state["hook"] = None
                return None
            lib.axon_start_nrt_profile.argtypes = [
                ctypes.POINTER(ctypes.c_int64), ctypes.c_size_t]
            lib.axon_start_nrt_profile.restype = ctypes.c_int64
            lib.axon_stop_nrt_profile.argtypes = [ctypes.c_char_p]
            lib.axon_stop_nrt_profile.restype = ctypes.c_int64
        except OSError:
            _state["hook"] = None
            return None

        @contextlib.contextmanager
        def _hook(output_dir, device_ids):
            import jax
            jax.devices()
            if device_ids:
                ids = (ctypes.c_int64 * len(device_ids))(*device_ids)
                rc = lib.axon_start_nrt_profile(ids, len(device_ids))
            else:
                rc = lib.axon_start_nrt_profile(None, 0)
            if rc != 0:
                raise RuntimeError(f"axon_start_nrt_profile rc={rc}")
            try:
                yield
            finally:
                n = lib.axon_stop_nrt_profile(str(output_dir).encode())
                print(f"ntff profile: {n} file(s) written to {output_dir}")

        _state["hook"] = _hook
        return _hook

    mod.get_axon_ntff_profile_hook = get_axon_ntff_profile_hook
    _sys.modules["antenv.axon_hooks"] = mod


def kernel(**inputs):
    from concourse.bass_utils import run_bass_kernel_spmd

    if os.environ.get("BASS_KERNEL_TRACE"):
        _install_ntff_shim()
        import concourse.bass_utils as _bu
        _bu.upload_artifacts = lambda tmpdir: f"file://{tmpdir}"

    if "nc" not in _COMPILED:
        _COMPILED["nc"] = build_kernel()
    nc = _COMPILED["nc"]
    in_maps = _shard_inputs(inputs)
    res = run_bass_kernel_spmd(nc, in_maps, core_ids=list(range(N_CORES)),
                               trace=bool(os.environ.get("BASS_KERNEL_TRACE")))
    _COMPILED["last_result"] = res
    out = np.zeros((B, S, H), dtype=np.float32)
    for c in range(N_CORES):
        b = c // 4
        q0 = (c % 4) * I_CORE
        out[b, q0:q0 + I_CORE] = np.asarray(
            res.results[c]["out"]).reshape(I_CORE, H)
    return out


# revision 28
# speedup vs baseline: 2.6599x; 1.3314x over previous
"""Distributed Trainium2 Bass kernel for BrosAttention.

B=2, S=1024, H=768, NH=12, DH=64:
  q,k,v = heads(hidden @ W.T + b)
  scores = q@k^T + einsum('bnid,bijd->bnij', q, bpe)   (bpe = bbox transposed)
  probs  = softmax(scores / 8)
  out    = LN(probs@v @ Wo.T + bo + hidden)

Sharding: 8 cores = 2 batches x 4 query-row blocks of 256 rows. Each core
reads only its 64MB slice of bbox_pos_emb, computes K/V for the full
sequence of its batch (duplicated 4x, cheaper than a collective here), and
writes a disjoint [256, 768] output slice. No collectives.

Layout: transposed scores (scoresT[j, i] per head) because the bias term
q.bpe needs d on partitions; bpe arrives [j, d] and is PE-transposed with
two query rows packed per [128, j] tile. The bias matmul (lhsT = q of one
row as a [64, 12] weight) runs 4 i's concurrently in the four 32-column
groups of the PE array; bias tiles are PE-transposed again into [j, (i,n)]
and added to QK^T psum tiles via a stride-12 AP. Softmax-over-partitions
uses ones-vector matmuls; probs stay unnormalized until after P@V.
"""

import os
import sys
import numpy as np

sys.path.insert(0, "/opt/trn_rl_repo")

B, S, H, NH, DH = 2, 1024, 768, 12, 64
EPS = 1e-12
P = 128
I_CORE = S * B // 8  # 256
N_CORES = 8

_COMPILED = {}


def build_kernel(s=S, i_core=I_CORE, h=H, nh=NH, dh=DH):
    from contextlib import ExitStack
    from concourse import bacc, bass, mybir, tile

    f32 = mybir.dt.float32
    bf16 = mybir.dt.bfloat16
    Alu = mybir.AluOpType
    Act = mybir.ActivationFunctionType
    AxisX = mybir.AxisListType.X

    SC = s // P          # 8 seq chunks
    HC = h // P          # 6 hidden chunks
    IH = i_core // 2     # 128 i's per half
    NDUO_H = IH // 4     # 32 duos per half
    JH = min(512, s)     # fp32 matmul N limit / psum bank
    NJH = s // JH        # 2
    HP = nh // 2         # 6 head pairs
    VH = h // 2          # 384

    nc = bacc.Bacc(None, target_bir_lowering=False, debug=False)

    bf16_ = mybir.dt.bfloat16
    d_hidT = nc.declare_dram_parameter("hidT", [HC, P, s], bf16_, isOutput=False)
    d_hidRT = nc.declare_dram_parameter("hidRT", [HC, P, i_core], bf16_, isOutput=False)
    d_hidR = nc.declare_dram_parameter("hid_rows", [i_core // P, P, h], f32, isOutput=False)
    d_bpe = nc.declare_dram_parameter("bpe", [i_core, dh, s], bf16_, isOutput=False)
    d_W = {w: nc.declare_dram_parameter(w + "T", [HC, P, h], bf16_, isOutput=False)
           for w in ("Wq", "Wk", "Wv", "Wo")}
    d_b = {bn: nc.declare_dram_parameter(bn, [1, h], f32, isOutput=False)
           for bn in ("bq", "bk", "bv", "bo", "ln_gamma", "ln_beta")}
    d_ident = nc.declare_dram_parameter("ident", [P, P], f32, isOutput=False)
    d_out = nc.declare_dram_parameter("out", [i_core // P, P, h], f32, isOutput=True)

    with tile.TileContext(nc) as tc, ExitStack() as ctx:
        # ------------- long-lived pools -------------
        const_p = ctx.enter_context(tc.tile_pool(name="const", bufs=1))
        stat_p = ctx.enter_context(tc.tile_pool(name="stat", bufs=1))
        ps128 = ctx.enter_context(
            tc.tile_pool(name="ps128", bufs=3, space=bass.MemorySpace.PSUM))
        ps512 = ctx.enter_context(
            tc.tile_pool(name="ps512", bufs=1, space=bass.MemorySpace.PSUM))
        psB = ctx.enter_context(
            tc.tile_pool(name="psB", bufs=2, space=bass.MemorySpace.PSUM))
        psS = ctx.enter_context(
            tc.tile_pool(name="psS", bufs=1, space=bass.MemorySpace.PSUM))
        psC = ctx.enter_context(
            tc.tile_pool(name="psC", bufs=1, space=bass.MemorySpace.PSUM))

        # ------------- constants -------------
        ident = const_p.tile([P, P], f32)
        nc.sync.dma_start(ident[:], d_ident[:])
        ones_col = const_p.tile([P, 1], f32)
        nc.vector.memset(ones_col[:], 1.0)
        ones_row = const_p.tile([1, s], f32)
        nc.vector.memset(ones_row[:], 1.0)
        eps_t = const_p.tile([P, 1], f32)
        nc.vector.memset(eps_t[:], EPS)
        zrow = const_p.tile([1, P], bf16)
        nc.vector.memset(zrow[:], 0.0)
        ident_bf = const_p.tile([P, P], bf16)
        nc.vector.tensor_copy(ident_bf[:], ident[:])
        ones_col_bf = const_p.tile([P, 1], bf16)
        nc.vector.memset(ones_col_bf[:], 1.0)
        ones_row_bf = const_p.tile([1, s], bf16)
        nc.vector.memset(ones_row_bf[:], 1.0)
        b_sb = {}
        b_bf = {}
        for bn in ("bq", "bk", "bv", "bo", "ln_gamma", "ln_beta"):
            b_sb[bn] = const_p.tile([1, h], f32, name=f"bias_{bn}")
            nc.sync.dma_start(b_sb[bn][:], d_b[bn][:])
            b_bf[bn] = const_p.tile([1, h], bf16, name=f"biasbf_{bn}")
            nc.vector.tensor_copy(b_bf[bn][:], b_sb[bn][:])

        bcast = {}
        for bn in ("ln_gamma", "ln_beta"):
            t = stat_p.tile([P, h], f32, name=f"bcast_{bn}")
            for c in range(HC):
                pbx = ps128.tile([P, P], f32, name="pt")
                nc.tensor.matmul(pbx[:], ones_row[:, 0:P],
                                 b_sb[bn][:, c * P:(c + 1) * P])
                nc.scalar.copy(t[:, c * P:(c + 1) * P], pbx[:])
            bcast[bn] = t

        # long-lived activations
        hidR = stat_p.tile([P, i_core // P, h], f32)
        nc.sync.dma_start(hidR[:], d_hidR[:].transpose([1, 0, 2]))
        WoT = stat_p.tile([P, HC, h], bf16)
        nc.sync.dma_start(WoT[:], d_W["Wo"][:].transpose([1, 0, 2]))
        qT128 = stat_p.tile([P, nh, i_core], bf16)  # q[n,i,:] at both 64-halves
        qPair = stat_p.tile([P, i_core // 2, 32], bf16)
        kT128 = stat_p.tile([P, HP, s], bf16)
        v_sb = stat_p.tile([P, SC, h], bf16)

        def pe_T(dst_ap, src_ap, copy_eng):
            bf = src_ap.dtype == bf16
            pt = ps128.tile([P, P], bf16 if bf else f32, name="pt")
            n = src_ap.shape[-1]
            nc.tensor.transpose(pt[0:n, :], src_ap,
                                ident_bf[:] if bf else ident[:])
            if copy_eng is nc.scalar:
                copy_eng.copy(dst_ap, pt[0:n, :])
            else:
                copy_eng.tensor_copy(dst_ap, pt[0:n, :])

        # ------------- phase 0 -------------
        with tc.tile_pool(name="early", bufs=1) as early_p:
            hidT = early_p.tile([P, HC, s], bf16)
            nc.sync.dma_start(hidT[:], d_hidT[:].transpose([1, 0, 2]))
            hidRT = early_p.tile([P, HC, i_core], bf16)
            nc.sync.dma_start(hidRT[:], d_hidRT[:].transpose([1, 0, 2]))

            def load_WT(w, dst):
                nc.sync.dma_start(dst[:], d_W[w][:].transpose([1, 0, 2]))
                return dst

            # Q projection (transposed): qT = Wq @ hidR^T + bq
            WqT = load_WT("Wq", early_p.tile([P, HC, h], bf16, name="WT_q"))
            for r in range(HC):
                pq = ps512.tile([P, JH], f32, name="big")
                for kc in range(HC):
                    nc.tensor.matmul(pq[:, 0:i_core],
                                     WqT[:, kc, r * P:(r + 1) * P],
                                     hidRT[:, kc, :], start=(kc == 0), stop=False)
                nc.tensor.matmul(pq[:, 0:i_core], b_bf["bq"][:, r * P:(r + 1) * P],
                                 ones_row_bf[:, 0:i_core], start=False, stop=True)
                for sub in range(2):
                    src = pq[sub * dh:(sub + 1) * dh, 0:i_core]
                    nc.vector.tensor_copy(qT128[0:dh, 2 * r + sub, :], src)
                    nc.vector.tensor_copy(qT128[dh:P, 2 * r + sub, :], src)

            # qPair[k, p, m]: block-diag bias weights: rows 0-63 =
            # q_{2p} in cols 0:12, rows 64-127 = q_{2p+1} in cols 12:24.
            nc.vector.memset(qPair[:], 0.0)
            nc.vector.tensor_copy(
                qPair[0:dh, :, 0:nh],
                qT128[0:dh, :, 0::2].transpose([0, 2, 1]))
            nc.vector.tensor_copy(
                qPair[dh:P, :, nh:2 * nh],
                qT128[dh:P, :, 1::2].transpose([0, 2, 1]))

            # K projection (transposed): kT = Wk @ hid^T + bk
            WkT = load_WT("Wk", early_p.tile([P, HC, h], bf16, name="WT_k"))
            for r in range(HC):
                for jh in range(NJH):
                    pk = ps512.tile([P, JH], f32, name="big")
                    for kc in range(HC):
                        nc.tensor.matmul(pk[:], WkT[:, kc, r * P:(r + 1) * P],
                                         hidT[:, kc, jh * JH:(jh + 1) * JH],
                                         start=(kc == 0), stop=False)
                    nc.tensor.matmul(pk[:], b_bf["bk"][:, r * P:(r + 1) * P],
                                     ones_row_bf[:, 0:JH], start=False, stop=True)
                    nc.vector.tensor_copy(
                        kT128[:, r, jh * JH:(jh + 1) * JH], pk[:])

            # V projection (natural): v = hid @ Wv^T + bv
            WvT = load_WT("Wv", early_p.tile([P, HC, h], bf16, name="WT_v"))
            for jc in range(SC):
                for vh in range(2):
                    pv = ps512.tile([P, JH], f32, name="big")
                    for kc in range(HC):
                        nc.tensor.matmul(pv[:, 0:VH],
                                         hidT[:, kc, jc * P:(jc + 1) * P],
                                         WvT[:, kc, vh * VH:(vh + 1) * VH],
                                         start=(kc == 0), stop=False)
                    nc.tensor.matmul(pv[:, 0:VH], ones_row_bf[:, 0:P],
                                     b_bf["bv"][:, vh * VH:(vh + 1) * VH],
                                     start=False, stop=True)
                    nc.vector.tensor_copy(v_sb[:, jc, vh * VH:(vh + 1) * VH],
                                          pv[:, 0:VH])


        # ------------- phases A+B -------------
        with tc.tile_pool(name="bpeT", bufs=4) as bpeT_p, \
             tc.tile_pool(name="bias4", bufs=1) as bias4_p, \
             tc.tile_pool(name="biasT", bufs=1) as biasT_p, \
             tc.tile_pool(name="sm", bufs=2) as sm_p, \
             tc.tile_pool(name="ctxp", bufs=1) as ctx_p, \
             tc.tile_pool(name="yp", bufs=1) as y_p:
            for half in range(2):
                i0h = half * IH
                # biasT[j, jc, duo*48 + 12*i4 + n]
                biasT = biasT_p.tile([P, SC, NDUO_H * 4 * nh], bf16)

                for octo in range(NDUO_H // 2):
                    pb_h = [psB.tile([P, JH], f32, name="pbh") for j in range(NJH)]
                    for c4 in range(4):
                        pair = octo * 4 + c4
                        iA = i0h + 2 * pair
                        bpeT = bpeT_p.tile([P, s], bf16)
                        nc.sync.dma_start(
                            bpeT[:], d_bpe[iA:iA + 2].rearrange("a b c -> (a b) c"))
                        lhs = qPair[:, (i0h // 2) + pair, 0:32]
                        for jh in range(NJH):
                            nc.tensor.matmul(
                                pb_h[jh][32 * c4:32 * c4 + 32, :], lhs,
                                bpeT[:, jh * JH:(jh + 1) * JH],
                                tile_position=(0, 32 * c4))
                    b4 = bias4_p.tile([P, s], bf16)
                    for jh in range(NJH):
                        nc.vector.tensor_copy(b4[:, jh * JH:(jh + 1) * JH],
                                              pb_h[jh][:])
                    for jc in range(SC):
                        ptb = ps128.tile([P, P], bf16, name="pt")
                        nc.tensor.transpose(ptb[:], b4[:, jc * P:(jc + 1) * P],
                                            ident_bf[:])
                        nc.vector.tensor_copy(
                            biasT[:, jc, octo * 8 * nh:(octo + 1) * 8 * nh
                                  ].rearrange("p (a b) -> p a b", a=4),
                            ptb[:].rearrange("p (a b) -> p a b", a=4)[:, :, 0:2 * nh])

                # ---- attention ----
                ctxT = ctx_p.tile([P, HP, IH], bf16)
                for hp in range(HP):
                    pctx = psC.tile([P, IH], f32, name="pctx")
                    for sub in range(2):
                        n = 2 * hp + sub
                        probsT = sm_p.tile([P, SC, IH], bf16)
                        psum_s = psS.tile([1, IH], f32)
                        for jc in range(SC):
                            pqk = ps128.tile([P, IH], f32, name="pt")
                            sb = sub * dh
                            nc.tensor.matmul(pqk[:],
                                             kT128[sb:sb + dh, hp, jc * P:(jc + 1) * P],
                                             qT128[sb:sb + dh, n, i0h:i0h + IH])
                            sE = sm_p.tile([P, IH], f32)
                            nc.vector.tensor_tensor(
                                sE[:], pqk[:],
                                biasT[:, jc, n:n + nh * (IH - 1) + 1:nh], Alu.add)
                            nc.scalar.activation(probsT[:, jc, :], sE[:],
                                                 Act.Exp, scale=0.125)
                            nc.tensor.matmul(psum_s[:], ones_col_bf[:],
                                             probsT[:, jc, :],
                                             start=(jc == 0), stop=(jc == SC - 1),
                                             skip_group_check=True)
                        rec = sm_p.tile([1, IH], f32)
                        nc.vector.reciprocal(rec[:], psum_s[:])
                        prec = ps128.tile([P, IH], f32, name="pt")
                        nc.tensor.matmul(prec[0:dh, :], ones_row[:, 0:dh], rec[:])
                        recB = sm_p.tile([dh, IH], f32)
                        nc.scalar.copy(recB[:], prec[0:dh, :])
                        for jc in range(SC):
                            nc.tensor.matmul(
                                pctx[sub * dh:(sub + 1) * dh, :],
                                v_sb[:, jc, n * dh:(n + 1) * dh],
                                probsT[:, jc, :],
                                start=(jc == 0), stop=(jc == SC - 1),
                                tile_position=(0, sub * dh),
                                skip_group_check=True)
                        nc.vector.tensor_tensor(
                            pctx[sub * dh:(sub + 1) * dh, :],
                            pctx[sub * dh:(sub + 1) * dh, :],
                            recB[:], Alu.mult)
                    nc.scalar.copy(ctxT[:, hp, :], pctx[:])

                # ---- O-proj + residual + LN ----
                pys = [ps512.tile([P, VH], f32, name="big") for j in range(2)]
                for vh in range(2):
                    for kc in range(HC):
                        nc.tensor.matmul(pys[vh][:], ctxT[:, kc, :],
                                         WoT[:, kc, vh * VH:(vh + 1) * VH],
                                         start=(kc == 0), stop=False)
                    nc.tensor.matmul(pys[vh][:], ones_row_bf[:, 0:P],
                                     b_bf["bo"][:, vh * VH:(vh + 1) * VH],
                                     start=False, stop=True)
                y = y_p.tile([P, h], f32)
                for vh in range(2):
                    nc.vector.tensor_tensor(y[:, vh * VH:(vh + 1) * VH],
                                            pys[vh][:],
                                            hidR[:, half, vh * VH:(vh + 1) * VH],
                                            Alu.add)
                mu = y_p.tile([P, 1], f32)
                nc.vector.tensor_reduce(mu[:], y[:], AxisX, Alu.add)
                nc.vector.tensor_scalar(mu[:], mu[:], 1.0 / h, None, Alu.mult)
                yc = y_p.tile([P, h], f32)
                nc.vector.tensor_scalar(yc[:], y[:], mu[:], None, Alu.subtract)
                ssq = y_p.tile([P, 1], f32)
                nc.scalar.activation(y[:], yc[:], Act.Square, accum_out=ssq[:])
                std = y_p.tile([P, 1], f32)
                nc.scalar.activation(std[:], ssq[:], Act.Sqrt,
                                     scale=1.0 / h, bias=eps_t[:])
                rstd = y_p.tile([P, 1], f32)
                nc.vector.reciprocal(rstd[:], std[:])
                o1 = y_p.tile([P, h], f32)
                nc.vector.tensor_scalar(o1[:], yc[:], rstd[:], None, Alu.mult)
                nc.vector.tensor_tensor(o1[:], o1[:], bcast["ln_gamma"][:], Alu.mult)
                nc.vector.tensor_tensor(o1[:], o1[:], bcast["ln_beta"][:], Alu.add)
                nc.sync.dma_start(d_out[half], o1[:])

    nc.compile()
    return nc


def _shard_inputs(inputs):
    import ml_dtypes
    bf = ml_dtypes.bfloat16
    hs = np.ascontiguousarray(np.asarray(inputs["hidden_states"]), dtype=np.float32)
    bpe = np.asarray(inputs["bbox_pos_emb"])
    ident = np.eye(P, dtype=np.float32)
    # per-batch transposed hidden [H, S] in bf16
    hsT = {b: np.ascontiguousarray(hs[b].T.astype(bf)).reshape(H // P, P, S)
           for b in range(B)}
    WT = {w: np.ascontiguousarray(
             np.asarray(inputs[w], dtype=np.float32).T.astype(bf)).reshape(
                 H // P, P, H)
          for w in ("Wq", "Wk", "Wv", "Wo")}
    in_maps = []
    for c in range(N_CORES):
        b = c // 4
        q0 = (c % 4) * I_CORE
        m = {
            "hidT": hsT[b],
            "hidRT": np.ascontiguousarray(
                hs[b, q0:q0 + I_CORE].T.astype(bf)).reshape(H // P, P, I_CORE),
            "hid_rows": np.ascontiguousarray(
                hs[b, q0:q0 + I_CORE].reshape(I_CORE // P, P, H)),
            "bpe": np.ascontiguousarray(
                bpe[q0:q0 + I_CORE, :, b, :].transpose(0, 2, 1).astype(bf)),
            "ident": ident,
        }
        for w in ("Wq", "Wk", "Wv", "Wo"):
            m[w + "T"] = WT[w]
        for bn in ("bq", "bk", "bv", "bo", "ln_gamma", "ln_beta"):
            m[bn] = np.ascontiguousarray(
                np.asarray(inputs[bn], dtype=np.float32).reshape(1, H))
        in_maps.append(m)
    return in_maps


def _install_ntff_shim():
    """The agent image's antenv lacks axon_hooks; recreate the NTFF profile
    hook via ctypes against libaxon_pjrt.so so trace=True yields
    exec_time_ns + a perfetto trace."""
    import sys as _sys
    if "antenv.axon_hooks" in _sys.modules:
        return
    import types, ctypes, contextlib
    so_path = "/opt/axon/libaxon_pjrt.so"
    mod = types.ModuleType("antenv.axon_hooks")
    _state = {}

    def get_axon_ntff_profile_hook():
        if "hook" in _state:
            return _state["hook"]
        try:
            lib = ctypes.CDLL(so_path)
            if not hasattr(lib, "axon_start_nrt_profile"):
                _state["hook"] = None
                return None
            lib.axon_start_nrt_profile.argtypes = [
                ctypes.POINTER(ctypes.c_int64), ctypes.c_size_t]
            lib.axon_start_nrt_profile.restype = ctypes.c_int64
            lib.axon_stop_nrt_profile.argtypes = [ctypes.c_char_p]
            lib.axon_stop_nrt_profile.restype = ctypes.c_int64
        except OSError:
            _state["hook"] = None
            return None

        @contextlib.contextmanager
        def _hook(output_dir, device_ids):
            import jax
            jax.devices()
            if device_ids:
                ids = (ctypes.c_int64 * len(device_ids))(*device_ids)
                rc = lib.axon_start_nrt_profile(ids, len(device_ids))
            else:
                rc = lib.axon_start_nrt_profile(None, 0)
            if rc != 0:
                raise RuntimeError(f"axon_start_nrt_profile rc={rc}")
            try:
                yield
            finally:
                n = lib.axon_stop_nrt_profile(str(output_dir).encode())
                print(f"ntff profile: {n} file(s) written to {output_dir}")

        _state["hook"] = _hook
        return _hook

    mod.get_axon_ntff_profile_hook = get_axon_ntff_profile_hook
    _sys.modules["antenv.axon_hooks"] = mod


def kernel(**inputs):
    from concourse.bass_utils import run_bass_kernel_spmd

    if os.environ.get("BASS_KERNEL_TRACE"):
        _install_ntff_shim()
        import concourse.bass_utils as _bu
        _bu.upload_artifacts = lambda tmpdir: f"file://{tmpdir}"

    if "nc" not in _COMPILED:
        _COMPILED["nc"] = build_kernel()
    nc = _COMPILED["nc"]
    in_maps = _shard_inputs(inputs)
    res = run_bass_kernel_spmd(nc, in_maps, core_ids=list(range(N_CORES)),
                               trace=bool(os.environ.get("BASS_KERNEL_TRACE")))
    _COMPILED["last_result"] = res
    out = np.zeros((B, S, H), dtype=np.float32)
    for c in range(N_CORES):
        b = c // 4
        q0 = (c % 4) * I_CORE
        out[b, q0:q0 + I_CORE] = np.asarray(
            res.results[c]["out"]).reshape(I_CORE, H)
    return out
